# revision 18
# baseline (speedup 1.0000x reference)
"""SuperGAT x15 Trainium2 kernel (8 NeuronCores, SPMD).

Self-contained: hardcodes all shapes. Strategy:
- Nodes permuted by "need" (balanced split degree), striped across 8 cores
  (core = rank % 8, pos = rank // 8). Each core owns 6250 nodes and all
  edges whose dst it owns.
- Per layer, each core holds a replicated DRAM table of rows
  [hp(32) bf16 | aL f32 | aR f32] = 36 bf16-slots = 72B at 256B stride.
- Messages gathered per edge-slot via dma_gather (int16 idxs). The int16
  range limit (32767) is handled with two overlapping table views:
  region A = rows [0, 32768), region B = rows [17232, 50000).
  Each node's in-edges are split between regions (balanced), padded to a
  per-block schedule Dh[b] shared by all cores (SPMD: one program).
- Layout C: node-per-partition, slots along free axis. Segment softmax =
  free-axis reductions. No per-edge scatter: aggregation output lands
  per-node directly.
- Per-layer exchange: own table rows -> DRAM bounce -> AllGather ->
  spread DMA into the 256B-stride gather table.
"""
import os
import hashlib
import numpy as np
import ml_dtypes

import concourse.bacc as bacc
import concourse.bass as bass
import concourse.tile as tile
from concourse import mybir, bass_utils, library_config
from concourse.masks import make_identity

dt = mybir.dt

# problem constants
N = 50000
E = 800000
D_IN = 128
H = 32
D_OUT = 16
L_FULL = 15
NEG = 0.2
NC = 8
NPC = N // NC            # 6250 nodes per core
NBLK = (NPC + 127) // 128  # 49 blocks
NPAD = NBLK * 128        # 6272 padded positions
T_HI = 32768
T_LO = N - T_HI          # 17232
ROWW = 36                # bf16 slots per table row (72B payload)
TABW = 128               # bf16 slots per table row stride (256B)

L_DEBUG = int(os.environ.get("SGAT_LAYERS", str(L_FULL)))
MAX_IDX_PER_GATHER = 16000
CHUNK_SLOTS = int(os.environ.get("SGAT_CHUNK", "120"))  # per-partition per-region
N_QUEUES = int(os.environ.get("SGAT_QUEUES", "4"))      # SWDGE rings to cycle


def _patch_dma_gather_assert():
    import inspect, textwrap
    if getattr(bass.BassGpSimd.dma_gather, "_sgat_patched", False):
        return
    src = inspect.getsource(bass.BassGpSimd.dma_gather)
    src = src.replace(
        "assert (\n            elem_size_bytes > 0 and elem_size_bytes % 256 == 0\n        )  # transpose restriction",
        "assert elem_size_bytes > 0")
    src = textwrap.dedent(src)
    ns = dict(bass.BassGpSimd.dma_gather.__globals__)
    exec(src, ns)
    fn = ns["dma_gather"]
    fn._sgat_patched = True
    bass.BassGpSimd.dma_gather = fn


_patch_dma_gather_assert()


# ----------------------------------------------------------------------------
# host-side graph preprocessing
# ----------------------------------------------------------------------------

def _preprocess(edge_index):
    src0 = edge_index[0].astype(np.int64)
    dst0 = edge_index[1].astype(np.int64)
    loops = np.arange(N, dtype=np.int64)
    src0 = np.concatenate([src0, loops])
    dst0 = np.concatenate([dst0, loops])

    deg = np.bincount(dst0, minlength=N)
    # Permutation sorted by degree (need ~ ceil(deg/2) + split imbalance;
    # the schedule below uses the ACTUAL per-node need, so correctness does
    # not depend on this ordering -- only padding efficiency does).
    rank_of = np.argsort(-deg, kind="stable")      # rank -> orig node
    r = np.arange(N, dtype=np.int64)
    pid_of_rank = (r % NC) * NPC + r // NC
    perm = np.empty(N, dtype=np.int64)             # orig -> permuted id
    perm[rank_of] = pid_of_rank
    inv_perm = np.empty(N, dtype=np.int64)         # permuted id -> orig
    inv_perm[perm] = np.arange(N, dtype=np.int64)

    psrc = perm[src0]
    pdst = perm[dst0]

    pdeg = np.bincount(pdst, minlength=N)          # per permuted node
    nAf = np.bincount(pdst[psrc < T_LO], minlength=N)
    nBf = np.bincount(pdst[psrc >= T_HI], minlength=N)
    need = np.maximum(np.maximum(nAf, nBf), (pdeg + 1) // 2)

    # block schedule: Dh[b] = max need over all cores' block b
    need_pad = np.zeros(NC * NPAD, dtype=np.int64)
    node_pid = np.arange(N)
    need_pad[(node_pid // NPC) * NPAD + node_pid % NPC] = need
    Dh = need_pad.reshape(NC, NBLK, 128).max(axis=(0, 2)).astype(np.int64)
    Dh = np.maximum(Dh, 1)
    # quantize Dh upward to a small ladder: longer equal-D runs merge into
    # fewer groups -> fewer (instruction-count-bound) vector ops per layer
    ladder = np.array([1, 2, 3, 4, 6, 8, 10, 12, 14, 16, 20, 24, 28, 32,
                       40, 48, 64, 96, 128], dtype=np.int64)
    Dh = ladder[np.searchsorted(ladder, Dh)]

    # group blocks with equal Dh, G*Dh <= CHUNK_SLOTS
    groups = []  # (b0, G, D)
    b = 0
    while b < NBLK:
        d = int(Dh[b])
        g = 1
        while (b + g < NBLK and Dh[b + g] == d
               and (g + 1) * d <= max(d, CHUNK_SLOTS)):
            g += 1
        groups.append((b, g, d))
        b += g
    # chunks: consecutive groups, per-region slots <= CHUNK_SLOTS and
    # idx count <= MAX_IDX_PER_GATHER
    chunks = []  # list of (group_lo, group_hi) indices into groups
    offq = np.zeros(NBLK, dtype=np.int64)  # per-block region-slot offset q
    q = 0
    for (b0, g, d) in groups:
        for bb in range(b0, b0 + g):
            offq[bb] = q
            q += d
    SA = int(q)  # per-partition slots per region
    # graded caps: small first chunks (fast post-exchange pipeline fill)
    def cap_for(ci):
        return (32, 64)[ci] if ci < 2 else CHUNK_SLOTS
    lo = 0
    while lo < len(groups):
        hi = lo
        s = 0
        cap = cap_for(len(chunks))
        while hi < len(groups):
            b0, g, d = groups[hi]
            add = g * d
            if s > 0 and (s + add > cap
                          or (s + add) * 128 > MAX_IDX_PER_GATHER):
                break
            s += add
            hi += 1
        chunks.append((lo, hi, s))
        lo = hi
    # split a fat trailing chunk so the exchange isn't gated on one big tail
    if len(chunks) > 1 and chunks[-1][2] > 64 and chunks[-1][1] - chunks[-1][0] > 1:
        glo, ghi, s = chunks.pop()
        mid = glo
        acc = 0
        while mid < ghi - 1 and acc + groups[mid][1] * groups[mid][2] < s // 2:
            acc += groups[mid][1] * groups[mid][2]
            mid += 1
        mid = max(mid, glo + 1)
        s1 = sum(g * d for (_, g, d) in groups[glo:mid])
        if 0 < s1 < s:
            chunks.append((glo, mid, s1))
            chunks.append((mid, ghi, s - s1))
        else:
            chunks.append((glo, ghi, s))

    # per-core slot tables
    # edge assignment: sort edges by (core, pos) then split regions per node
    eorder = np.lexsort((psrc, pdst))
    s_src = psrc[eorder]
    s_dst = pdst[eorder]
    starts = np.searchsorted(s_dst, np.arange(N))
    ends = np.searchsorted(s_dst, np.arange(N) + 1)

    idxA = np.zeros((NC, 128, SA), dtype=np.int16)
    idxB = np.zeros((NC, 128, SA), dtype=np.int16)
    maskA = np.full((NC, 128, SA), -1e30, dtype=np.float32)
    maskB = np.full((NC, 128, SA), -1e30, dtype=np.float32)

    for n in range(N):
        e0, e1 = starts[n], ends[n]
        if e0 == e1:
            continue
        ss = s_src[e0:e1]
        c = n // NPC
        p = n % NPC
        bb = p // 128
        pp = p % 128
        d = int(Dh[bb])
        q0 = int(offq[bb])
        fa = ss[ss < T_LO]
        fb = ss[ss >= T_HI]
        fx = ss[(ss >= T_LO) & (ss < T_HI)]
        na, nb, nd = len(fa), len(fb), len(ss)
        lo_t = max(na, nd - d)
        hi_t = min(na + len(fx), d)
        ta = min(max((nd + 1) // 2, lo_t), hi_t)
        a_list = np.concatenate([fa, fx[: ta - na]])
        b_list = np.concatenate([fb, fx[ta - na:]])
        la, lb = len(a_list), len(b_list)
        assert la <= d and lb <= d, (n, la, lb, d)
        idxA[c, pp, q0:q0 + la] = a_list.astype(np.int16)
        maskA[c, pp, q0:q0 + la] = 0.0
        idxB[c, pp, q0:q0 + lb] = (b_list - T_LO).astype(np.int16)
        maskB[c, pp, q0:q0 + lb] = 0.0

    # wrap idxs for dma_gather: position i = q*128 + p -> [i%16, i//16], x8
    def wrap(idx):  # [128, SA] -> [128, SA*8] int16
        # i = q*128 + p ; element at [i % 16, i // 16]
        flat = idx.transpose(1, 0).reshape(-1)          # i-major
        w16 = flat.reshape(-1, 16).T                    # [16, SA*8]
        return np.tile(w16, (8, 1)).astype(np.int16)

    idxA_w = np.stack([wrap(idxA[c]) for c in range(NC)])
    idxB_w = np.stack([wrap(idxB[c]) for c in range(NC)])
    mask = np.stack([np.concatenate([maskA[c], maskB[c]], axis=1)
                     for c in range(NC)])               # [NC, 128, 2*SA]

    sched = dict(Dh=Dh, groups=groups, chunks=chunks, offq=offq, SA=SA)
    key = hashlib.sha256(
        (str(groups) + str(chunks) + str(L_DEBUG) + os.environ.get("SGAT_DUMP", "")).encode()).hexdigest()[:16]
    return dict(perm=perm, inv_perm=inv_perm, sched=sched, key=key,
                idxA=idxA_w, idxB=idxB_w, mask=mask)


# ----------------------------------------------------------------------------
# weights preprocessing
# ----------------------------------------------------------------------------

def _prep_weights(W0, b0, Ws, att_l, att_r, bs, W16, b16):
    # table_1 = (x @ W0 + b0) @ W1aug ; W1aug = [W1 | W1@al1 | W1@ar1]
    def aug(Wl, al, ar):
        A = np.zeros((H, ROWW), np.float32)
        A[:, :H] = Wl
        A[:, H] = Wl @ al
        A[:, H + 1] = Wl @ ar
        return A

    W1aug = aug(Ws[0], att_l[0], att_r[0])
    wfold = (W0 @ W1aug).astype(np.float32)            # [128, 36]
    bfold = (b0 @ W1aug).astype(np.float32)            # [36]
    waug = np.zeros((L_FULL, H, ROWW), np.float32)
    for l in range(1, L_FULL):
        waug[l - 1] = aug(Ws[l], att_l[l], att_r[l])
    waug[L_FULL - 1, :, :D_OUT] = W16                  # layer-15 tail
    brep = np.tile(bs[:, None, :], (1, 128, 1)).astype(np.float32)
    bfold_rep = np.tile(bfold[None, :], (128, 1)).astype(np.float32)
    b16rep = np.tile(b16[None, :], (128, 1)).astype(np.float32)
    return dict(wfold=wfold, bfold=bfold_rep, waug=waug, brep=brep,
                b16rep=b16rep)


# ----------------------------------------------------------------------------
# program builder
# ----------------------------------------------------------------------------

def _build_program(sched, sim=False):
    groups = sched["groups"]
    chunks = sched["chunks"]
    offq = sched["offq"]
    SA = sched["SA"]
    LN = L_DEBUG

    nc = bacc.Bacc(num_devices=NC, num_swdge_queues=N_QUEUES)
    xT_in = nc.dram_tensor("xT", [D_IN, NPAD], dt.float32, kind="ExternalInput")
    idxA_in = nc.dram_tensor("idxA", [128, SA * 8], dt.int16, kind="ExternalInput")
    idxB_in = nc.dram_tensor("idxB", [128, SA * 8], dt.int16, kind="ExternalInput")
    mask_in = nc.dram_tensor("mask", [128, 2 * SA], dt.float32, kind="ExternalInput")
    wfold_in = nc.dram_tensor("wfold", [D_IN, ROWW], dt.float32, kind="ExternalInput")
    bfold_in = nc.dram_tensor("bfold", [128, ROWW], dt.float32, kind="ExternalInput")
    waug_in = nc.dram_tensor("waug", [L_FULL, H, ROWW], dt.float32, kind="ExternalInput")
    brep_in = nc.dram_tensor("brep", [L_FULL, 128, H], dt.float32, kind="ExternalInput")
    b16_in = nc.dram_tensor("b16rep", [128, D_OUT], dt.float32, kind="ExternalInput")

    qn_state = [0]

    def next_queue():
        q = qn_state[0]
        qn_state[0] = (q + 1) % N_QUEUES
        return q

    DUMP = os.environ.get("SGAT_DUMP", "")
    if DUMP == "table":
        out_d = nc.dram_tensor("out", [N, ROWW], dt.uint16, kind="ExternalOutput")
    elif DUMP == "gbuf":
        csl0 = chunks[0][2]
        out_d = nc.dram_tensor("out", [128, 2 * csl0 * ROWW], dt.uint16,
                               kind="ExternalOutput")
    elif DUMP in ("alpha", "ex", "agg"):
        csl0 = chunks[0][2]
        w = csl0 * H if DUMP == "agg" else csl0
        out_d = nc.dram_tensor("out", [128, 2 * w], dt.float32,
                               kind="ExternalOutput")
    elif LN >= L_FULL:
        out_d = nc.dram_tensor("out", [NPAD, D_OUT], dt.float32, kind="ExternalOutput")
    else:
        out_d = nc.dram_tensor("out", [NPAD, ROWW], dt.uint16, kind="ExternalOutput")

    with tile.TileContext(nc) as tc:
        with tc.tile_pool(name="res", bufs=1) as res, \
             tc.tile_pool(name="gp", bufs=3) as gp, \
             tc.tile_pool(name="wp", bufs=2) as wp, \
             tc.tile_pool(name="sp", bufs=3) as sp, \
             tc.tile_pool(name="tp", bufs=2) as tp, \
             tc.tile_pool(name="pt", bufs=2, space="PSUM") as pt, \
             tc.tile_pool(name="pm", bufs=2, space="PSUM") as pm, \
             tc.tile_pool(name="dram", bufs=2, space="DRAM") as dram:

            nc.gpsimd.load_library(library_config.mlp)

            # residents
            xT = res.tile([D_IN, NPAD], dt.float32)
            nc.sync.dma_start(out=xT[:], in_=xT_in[:])
            idxA = res.tile([128, SA * 8], dt.int16)
            nc.sync.dma_start(out=idxA[:], in_=idxA_in[:])
            idxB = res.tile([128, SA * 8], dt.int16)
            nc.sync.dma_start(out=idxB[:], in_=idxB_in[:])
            maskr = res.tile([128, 2 * SA], dt.float32)
            nc.sync.dma_start(out=maskr[:], in_=mask_in[:])
            wfold = res.tile([D_IN, ROWW], dt.float32)
            nc.sync.dma_start(out=wfold[:], in_=wfold_in[:])
            bfold = res.tile([128, ROWW], dt.float32)
            nc.sync.dma_start(out=bfold[:], in_=bfold_in[:])
            waug = res.tile([H, L_FULL * ROWW], dt.float32)
            nc.sync.dma_start(
                out=waug[:].rearrange("h (l w) -> h l w", l=L_FULL),
                in_=waug_in[:].rearrange("l h w -> h l w"))
            brep = res.tile([128, L_FULL * H], dt.float32)
            nc.sync.dma_start(
                out=brep[:].rearrange("p (l h) -> p l h", l=L_FULL),
                in_=brep_in[:].rearrange("l p h -> p l h"))
            b16r = res.tile([128, D_OUT], dt.float32)
            nc.sync.dma_start(out=b16r[:], in_=b16_in[:])
            ident = res.tile([128, 128], dt.float32)
            make_identity(nc, ident[:])

            own_tabs = [res.tile([128, NBLK, ROWW], dt.bfloat16, name=f"own{i}")
                        for i in range(2)]
            outstage = res.tile([128, NBLK, D_OUT], dt.float32)

            def pack_row(psum_ap, own_tab, b):
                # psum [128, 36] f32 -> own_tab[:, b, :] (hp bf16 + aL/aR f32)
                bf = own_tab[:]
                nc.vector.tensor_copy(out=bf[:, b, 0:H], in_=psum_ap[:, 0:H])
                f32v = own_tab[:].bitcast(dt.float32)
                nc.scalar.copy(out=f32v[:, b, H // 2:H // 2 + 2],
                               in_=psum_ap[:, H:H + 2])

            # ---------------- conv0 + fold into table_1 -----------------
            own = own_tabs[0]
            for b in range(NBLK):
                ps = pm.tile([128, ROWW], dt.float32, space="PSUM", tag="mm")
                nc.tensor.matmul(out=ps[:], lhsT=xT[:, b * 128:(b + 1) * 128],
                                 rhs=wfold[:], start=True, stop=True)
                ps2 = sp.tile([128, ROWW], dt.float32, tag="c0add")
                nc.vector.tensor_tensor(out=ps2[:], in0=ps[:], in1=bfold[:],
                                        op=mybir.AluOpType.add)
                pack_row(ps2[:], own, b)

            def exchange(own_tab, li):
                bounce = dram.tile([NPAD, ROWW], dt.bfloat16, tag="bounce")
                bv = bounce[:].rearrange("(b p) w -> p b w", p=128)
                bsplit = [0, 12, 24, 36, NBLK]
                for si in range(4):
                    s0, s1 = bsplit[si], bsplit[si + 1]
                    nc.sync.dma_start(out=bv[:, s0:s1],
                                      in_=own_tab[:, s0:s1])
                table = dram.tile([N, TABW], dt.bfloat16, tag="table")
                if sim:
                    # timing-equivalent stand-in for AllGather + spread
                    for c in range(NC):
                        nc.sync.dma_start(
                            out=table[c * NPC:(c + 1) * NPC, 0:ROWW],
                            in_=bounce[0:NPC, :])
                    return table
                agout = dram.tile([N, ROWW], dt.bfloat16, tag="agout")
                nc.gpsimd.collective_compute(
                    "AllGather", mybir.AluOpType.bypass,
                    replica_groups=[list(range(NC))],
                    ins=[bounce[0:NPC, :]], outs=[agout[:]])
                nc.sync.dma_start(out=table[:, 0:ROWW], in_=agout[:])
                return table

            if LN == 0 and not DUMP:
                nc.sync.dma_start(
                    out=out_d[:].rearrange("(b p) w -> p b w", p=128),
                    in_=own[:].bitcast(dt.uint16))
            table = exchange(own, 0)
            if DUMP == "table":
                nc.sync.dma_start(out=out_d[:],
                                  in_=table[:, 0:ROWW].bitcast(dt.uint16))
            elif DUMP == "gbuf":
                glo, ghi, csl = chunks[0]
                q0 = int(offq[groups[glo][0]])
                gb = gp.tile([128, 2, csl, ROWW], dt.bfloat16, tag="gb")
                for r in range(2):
                    tab_view = table[0:T_HI, 0:ROWW] if r == 0 \
                        else table[T_LO:N, 0:ROWW]
                    idxr = idxA if r == 0 else idxB
                    nidx = csl * 128
                    nc.gpsimd.dma_gather(
                        out_ap=gb[:, r, :, :], in_ap=tab_view,
                        idxs_ap=idxr[:, q0 * 8:(q0 + csl) * 8],
                        num_idxs=nidx, num_idxs_reg=nidx,
                        elem_size=ROWW, elem_step=TABW, single_packet=False,
                        queue_num=next_queue())
                nc.sync.dma_start(
                    out=out_d[:],
                    in_=gb[:].rearrange("p r q w -> p (r q w)").bitcast(dt.uint16))
            if DUMP in ("table", "gbuf"):
                LN_eff = 0
            elif DUMP:
                LN_eff = 1
            else:
                LN_eff = LN
            dbg = None
            if DUMP in ("alpha", "ex", "agg"):
                _w = chunks[0][2] * (H if DUMP == "agg" else 1)
                dbg = res.tile([128, 2 * _w], dt.float32, name="dbg")

            # ---------------- layers ----------------
            for li in range(1, LN_eff + 1):
                own_prev = own_tabs[(li + 1) % 2]
                own_new = own_tabs[li % 2]
                last = (li == L_FULL)
                for (glo, ghi, csl) in chunks:
                    b0 = groups[glo][0]
                    q0 = int(offq[b0])
                    gb = gp.tile([128, 2, csl, ROWW], dt.bfloat16, tag="gb")
                    for r in range(2):
                        tab_view = table[0:T_HI, 0:ROWW] if r == 0 \
                            else table[T_LO:N, 0:ROWW]
                        idxr = idxA if r == 0 else idxB
                        nidx = csl * 128
                        nc.gpsimd.dma_gather(
                            out_ap=gb[:, r, :, :],
                            in_ap=tab_view,
                            idxs_ap=idxr[:, q0 * 8:(q0 + csl) * 8],
                            num_idxs=nidx, num_idxs_reg=nidx,
                            elem_size=ROWW, elem_step=TABW,
                            single_packet=False, queue_num=next_queue())
                    for gi in range(glo, ghi):
                        bg, G, D = groups[gi]
                        qa = int(offq[bg]) - q0
                        GD = G * D
                        S2 = 2 * GD
                        # views
                        hp_g = gb[:, :, qa:qa + GD, 0:H].rearrange(
                            "p r (g d) f -> p r g d f", g=G)
                        gf32 = gb[:].bitcast(dt.float32)
                        aL_g = gf32[:, :, qa:qa + GD, H // 2]       # [p,2,GD]
                        ownf = own_prev[:].bitcast(dt.float32)
                        aR_o = ownf[:, bg:bg + G, H // 2 + 1]       # [p,G]
                        hp_o = own_prev[:, bg:bg + G, 0:H]          # [p,G,32]

                        prod = wp.tile([128, S2, H], dt.bfloat16, tag="prod")
                        prodv = prod[:].rearrange("p (r q) f -> p r q f", r=2)
                        for r in range(2):
                            nc.vector.tensor_tensor(
                                out=prodv[:, r].rearrange(
                                    "p (g d) f -> p g d f", g=G),
                                in0=gb[:, r, qa:qa + GD, 0:H].rearrange(
                                    "p (g d) f -> p g d f", g=G),
                                in1=hp_o.unsqueeze(2)
                                .broadcast_to([128, G, D, H]),
                                op=mybir.AluOpType.mult)
                        logit = sp.tile([128, S2], dt.float32, tag="logit")
                        nc.vector.tensor_reduce(
                            out=logit[:], in_=prod[:],
                            axis=mybir.AxisListType.X, op=mybir.AluOpType.add,
                            negate=True)
                        sig = sp.tile([128, S2], dt.float32, tag="sig")
                        nc.scalar.activation(
                            out=sig[:], in_=logit[:],
                            func=mybir.ActivationFunctionType.Exp)
                        nc.vector.tensor_scalar(
                            out=sig[:], in0=sig[:], scalar1=1.0, scalar2=None,
                            op0=mybir.AluOpType.add)
                        nc.vector.reciprocal(out=sig[:], in_=sig[:])
                        alpha = sp.tile([128, S2], dt.float32, tag="alpha")
                        nc.vector.tensor_tensor(
                            out=alpha[:].rearrange("p (r g d) -> p r g d",
                                                   r=2, g=G),
                            in0=aL_g.rearrange("p r (g d) -> p r g d", g=G),
                            in1=aR_o.unsqueeze(1).unsqueeze(3).broadcast_to(
                                [128, 2, G, D]),
                            op=mybir.AluOpType.add)
                        nc.vector.tensor_tensor(out=alpha[:], in0=alpha[:],
                                                in1=sig[:],
                                                op=mybir.AluOpType.mult)
                        asc = sp.tile([128, S2], dt.float32, tag="asc")
                        nc.vector.tensor_scalar(
                            out=asc[:], in0=alpha[:], scalar1=NEG, scalar2=None,
                            op0=mybir.AluOpType.mult)
                        nc.vector.tensor_tensor(
                            out=alpha[:], in0=alpha[:], in1=asc[:],
                            op=mybir.AluOpType.max)
                        mk = maskr[:].rearrange("p (r q) -> p r q", r=2)[
                            :, :, qa + q0:qa + q0 + GD]
                        nc.vector.tensor_tensor(
                            out=alpha[:].rearrange("p (r q) -> p r q", r=2),
                            in0=alpha[:].rearrange("p (r q) -> p r q", r=2),
                            in1=mk, op=mybir.AluOpType.add)
                        if dbg is not None and li == 1 and glo == 0 and DUMP == "alpha":
                            nc.vector.tensor_copy(
                                out=dbg[:].rearrange("p (r q) -> p r q", r=2)[
                                    :, :, qa:qa + GD],
                                in_=alpha[:].rearrange("p (r q) -> p r q", r=2))
                        am2 = sp.tile([128, 2 * G], dt.float32, tag="am2")
                        nc.vector.tensor_reduce(
                            out=am2[:],
                            in_=alpha[:].rearrange("p (rg d) -> p rg d", d=D),
                            axis=mybir.AxisListType.X, op=mybir.AluOpType.max)
                        nam = sp.tile([128, G], dt.float32, tag="nam")
                        nc.vector.tensor_reduce(
                            out=nam[:],
                            in_=am2[:].rearrange("p (r g) -> p g r", r=2),
                            axis=mybir.AxisListType.X, op=mybir.AluOpType.max,
                            negate=True)
                        nc.vector.tensor_tensor(
                            out=alpha[:].rearrange("p (r g d) -> p r g d",
                                                   r=2, g=G),
                            in0=alpha[:].rearrange("p (r g d) -> p r g d",
                                                   r=2, g=G),
                            in1=nam[:].unsqueeze(1).unsqueeze(3).broadcast_to(
                                [128, 2, G, D]),
                            op=mybir.AluOpType.add)
                        ex = sp.tile([128, S2], dt.bfloat16, tag="ex")
                        nc.scalar.activation(
                            out=ex[:], in_=alpha[:],
                            func=mybir.ActivationFunctionType.Exp)
                        if dbg is not None and li == 1 and glo == 0 and DUMP == "ex":
                            nc.vector.tensor_copy(
                                out=dbg[:].rearrange("p (r q) -> p r q", r=2)[
                                    :, :, qa:qa + GD],
                                in_=ex[:].rearrange("p (r q) -> p r q", r=2))
                        den2 = sp.tile([128, 2 * G], dt.float32, tag="den2")
                        nc.vector.tensor_reduce(
                            out=den2[:],
                            in_=ex[:].rearrange("p (rg d) -> p rg d", d=D),
                            axis=mybir.AxisListType.X, op=mybir.AluOpType.add)
                        rden = sp.tile([128, G], dt.float32, tag="rden")
                        den1 = sp.tile([128, G], dt.float32, tag="den1")
                        nc.vector.tensor_reduce(
                            out=den1[:],
                            in_=den2[:].rearrange("p (r g) -> p g r", r=2),
                            axis=mybir.AxisListType.X, op=mybir.AluOpType.add)
                        nc.vector.reciprocal(out=rden[:], in_=den1[:])
                        wv = wp.tile([128, S2, H], dt.bfloat16, tag="wv")
                        nc.vector.tensor_tensor(
                            out=wv[:].rearrange("p (r q) f -> p r q f", r=2),
                            in0=gb[:, :, qa:qa + GD, 0:H],
                            in1=ex[:].rearrange("p (r q) -> p r q", r=2)
                            .unsqueeze(3).broadcast_to([128, 2, GD, H]),
                            op=mybir.AluOpType.mult)
                        agg2 = tp.tile([128, 2, G, H], dt.float32, tag="agg2")
                        nc.vector.tensor_reduce(
                            out=agg2[:].rearrange("p r g f -> p (r g) f"),
                            in_=wv[:].rearrange(
                                "p (r g d) f -> p (r g) f d", r=2, g=G),
                            axis=mybir.AxisListType.X,
                            op=mybir.AluOpType.add)
                        agg = tp.tile([128, G, H], dt.float32, tag="agg")
                        nc.vector.tensor_tensor(
                            out=agg[:], in0=agg2[:, 0], in1=agg2[:, 1],
                            op=mybir.AluOpType.add)
                        nc.vector.tensor_tensor(
                            out=agg[:], in0=agg[:],
                            in1=rden[:].unsqueeze(2).broadcast_to([128, G, H]),
                            op=mybir.AluOpType.mult)
                        if dbg is not None and li == 1 and glo == 0 and DUMP == "agg":
                            nc.vector.tensor_copy(
                                out=dbg[:, qa * H:(qa + G) * H],
                                in_=agg[:].rearrange("p g h -> p (g h)"))
                        nc.vector.tensor_tensor(
                            out=agg[:], in0=agg[:],
                            in1=brep[:].rearrange("p (l h) -> p l h",
                                                  l=L_FULL)[:, li - 1]
                            .unsqueeze(1).broadcast_to([128, G, H]),
                            op=mybir.AluOpType.add)
                        hnext = tp.tile([128, G, H], dt.float32, tag="hnext")
                        nc.vector.tensor_scalar(
                            out=hnext[:], in0=agg[:], scalar1=0.0, scalar2=None,
                            op0=mybir.AluOpType.max)
                        # tails per block
                        wslice = waug[:].rearrange(
                            "h (l w) -> h l w", l=L_FULL)[:, li - 1, :]
                        for gg in range(G):
                            b = bg + gg
                            hT_ps = pt.tile([H, 128], dt.float32,
                                            space="PSUM", tag="hT")
                            nc.tensor.transpose(out=hT_ps[:],
                                                in_=hnext[:, gg, :],
                                                identity=ident[:])
                            hT = sp.tile([H, 128], dt.float32, tag="hTs")
                            nc.scalar.copy(out=hT[:], in_=hT_ps[:])
                            mm = pm.tile([128, ROWW], dt.float32,
                                         space="PSUM", tag="mm")
                            if last:
                                nc.tensor.matmul(out=mm[:, 0:D_OUT],
                                                 lhsT=hT[:],
                                                 rhs=wslice[:, 0:D_OUT],
                                                 start=True, stop=True)
                                nc.vector.tensor_tensor(
                                    out=outstage[:, b, :],
                                    in0=mm[:, 0:D_OUT], in1=b16r[:],
                                    op=mybir.AluOpType.add)
                            else:
                                nc.tensor.matmul(out=mm[:], lhsT=hT[:],
                                                 rhs=wslice[:],
                                                 start=True, stop=True)
                                pack_row(mm[:], own_new, b)
                if dbg is not None and li == 1:
                    nc.sync.dma_start(out=out_d[:], in_=dbg[:])
                    break
                if last:
                    nc.sync.dma_start(
                        out=out_d[:].rearrange("(b p) w -> p b w", p=128),
                        in_=outstage[:])
                elif li == LN:
                    nc.sync.dma_start(
                        out=out_d[:].rearrange("(b p) w -> p b w", p=128),
                        in_=own_new[:].bitcast(dt.uint16))
                else:
                    table = exchange(own_new, li)

    nc.compile()
    return nc


# ----------------------------------------------------------------------------
# entry point
# ----------------------------------------------------------------------------

_CACHE = {}
LAST_RESULT = None


def kernel(x, edge_index, W0, b0, Ws, att_l, att_r, bs, W16, b16):
    global LAST_RESULT
    x = np.asarray(x, dtype=np.float32)
    edge_index = np.asarray(edge_index)
    pre = _preprocess(edge_index)
    wts = _prep_weights(np.asarray(W0, np.float32), np.asarray(b0, np.float32),
                        np.asarray(Ws, np.float32),
                        np.asarray(att_l, np.float32),
                        np.asarray(att_r, np.float32),
                        np.asarray(bs, np.float32),
                        np.asarray(W16, np.float32),
                        np.asarray(b16, np.float32))
    key = pre["key"]
    if key not in _CACHE:
        _CACHE[key] = _build_program(pre["sched"])
    nc = _CACHE[key]

    inv_perm = pre["inv_perm"]
    in_maps = []
    for c in range(NC):
        pids = np.arange(c * NPC, (c + 1) * NPC)
        orig = inv_perm[pids]
        xT = np.zeros((D_IN, NPAD), np.float32)
        xT[:, 0:NPC] = x[orig].T
        in_maps.append(dict(
            xT=xT, idxA=pre["idxA"][c], idxB=pre["idxB"][c],
            mask=pre["mask"][c].reshape(128, -1),
            wfold=wts["wfold"], bfold=wts["bfold"], waug=wts["waug"],
            brep=wts["brep"], b16rep=wts["b16rep"]))

    res = bass_utils.run_bass_kernel_spmd(
        nc, in_maps, core_ids=list(range(NC)),
        tmpdir=os.environ.get("SGAT_TMPDIR") or None)
    LAST_RESULT = res

    if L_DEBUG >= L_FULL:
        out = np.zeros((N, D_OUT), np.float32)
        for c in range(NC):
            pids = np.arange(c * NPC, (c + 1) * NPC)
            out[inv_perm[pids]] = res.results[c]["out"][0:NPC]
        return out
    else:
        # debug: return raw table_{L+1} rows per permuted id
        out = np.zeros((N, ROWW), np.uint16)
        for c in range(NC):
            pids = np.arange(c * NPC, (c + 1) * NPC)
            out[inv_perm[pids]] = res.results[c]["out"][0:NPC]
        return out



# revision 21
# speedup vs baseline: 1.1078x; 1.1078x over previous
"""SuperGAT x15 Trainium2 kernel (8 NeuronCores, SPMD).

Self-contained: hardcodes all shapes. Strategy:
- Nodes permuted by "need" (balanced split degree), striped across 8 cores
  (core = rank % 8, pos = rank // 8). Each core owns 6250 nodes and all
  edges whose dst it owns.
- Per layer, each core holds a replicated DRAM table of rows
  [hp(32) bf16 | aL f32 | aR f32] = 36 bf16-slots = 72B at 256B stride.
- Messages gathered per edge-slot via dma_gather (int16 idxs). The int16
  range limit (32767) is handled with two overlapping table views:
  region A = rows [0, 32768), region B = rows [17232, 50000).
  Each node's in-edges are split between regions (balanced), padded to a
  per-block schedule Dh[b] shared by all cores (SPMD: one program).
- Layout C: node-per-partition, slots along free axis. Segment softmax =
  free-axis reductions. No per-edge scatter: aggregation output lands
  per-node directly.
- Per-layer exchange: own table rows -> DRAM bounce -> AllGather ->
  spread DMA into the 256B-stride gather table.
"""
import os
import hashlib
import numpy as np
import ml_dtypes

import concourse.bacc as bacc
import concourse.bass as bass
import concourse.tile as tile
from concourse import mybir, bass_utils, library_config
from concourse.masks import make_identity

dt = mybir.dt

# problem constants
N = 50000
E = 800000
D_IN = 128
H = 32
D_OUT = 16
L_FULL = 15
NEG = 0.2
NC = 8
NPC = N // NC            # 6250 nodes per core
NBLK = (NPC + 127) // 128  # 49 blocks
NPAD = NBLK * 128        # 6272 padded positions
T_HI = 32768
T_LO = N - T_HI          # 17232
ROWW = 36                # bf16 slots per table row (72B payload)
TABW = 128               # bf16 slots per table row stride (256B)

L_DEBUG = int(os.environ.get("SGAT_LAYERS", str(L_FULL)))
MAX_IDX_PER_GATHER = 16000
CHUNK_SLOTS = int(os.environ.get("SGAT_CHUNK", "120"))  # per-partition per-region
N_QUEUES = int(os.environ.get("SGAT_QUEUES", "4"))      # SWDGE rings to cycle


def _patch_dma_gather_assert():
    import inspect, textwrap
    if getattr(bass.BassGpSimd.dma_gather, "_sgat_patched", False):
        return
    src = inspect.getsource(bass.BassGpSimd.dma_gather)
    src = src.replace(
        "assert (\n            elem_size_bytes > 0 and elem_size_bytes % 256 == 0\n        )  # transpose restriction",
        "assert elem_size_bytes > 0")
    src = textwrap.dedent(src)
    ns = dict(bass.BassGpSimd.dma_gather.__globals__)
    exec(src, ns)
    fn = ns["dma_gather"]
    fn._sgat_patched = True
    bass.BassGpSimd.dma_gather = fn


_patch_dma_gather_assert()


# ----------------------------------------------------------------------------
# host-side graph preprocessing
# ----------------------------------------------------------------------------

def _preprocess(edge_index):
    src0 = edge_index[0].astype(np.int64)
    dst0 = edge_index[1].astype(np.int64)
    loops = np.arange(N, dtype=np.int64)
    src0 = np.concatenate([src0, loops])
    dst0 = np.concatenate([dst0, loops])

    deg = np.bincount(dst0, minlength=N)
    # Permutation sorted by degree (need ~ ceil(deg/2) + split imbalance;
    # the schedule below uses the ACTUAL per-node need, so correctness does
    # not depend on this ordering -- only padding efficiency does).
    rank_of = np.argsort(-deg, kind="stable")      # rank -> orig node
    # Band-swap: within each 8-rank stratum (one node per core, same blocks),
    # send the two highest OUT-degree nodes to cores 3 and 4 -- their whole
    # PID range lies inside the int16 overlap band [T_LO, T_HI), so more
    # edges become region-flexible and the padded schedule shrinks.
    odeg = np.bincount(src0, minlength=N)          # out-degree (incl loop)
    ro = rank_of[: (N // NC) * NC].reshape(-1, NC)  # [stratum, 8 nodes]
    od = odeg[ro]
    order = np.argsort(-od, axis=1, kind="stable")  # per-stratum by out-deg
    # core slots ordered by in-band preference: 3,4 fully in band; 2,5 partly
    slot_pref = np.array([3, 4, 2, 5, 1, 6, 0, 7])
    new_ro = np.empty_like(ro)
    new_ro[np.arange(len(ro))[:, None], slot_pref[None, :]] = np.take_along_axis(
        ro, order, axis=1)
    rank_of = rank_of.copy()
    rank_of[: len(ro) * NC] = new_ro.reshape(-1)
    r = np.arange(N, dtype=np.int64)
    pid_of_rank = (r % NC) * NPC + r // NC
    perm = np.empty(N, dtype=np.int64)             # orig -> permuted id
    perm[rank_of] = pid_of_rank
    inv_perm = np.empty(N, dtype=np.int64)         # permuted id -> orig
    inv_perm[perm] = np.arange(N, dtype=np.int64)

    psrc = perm[src0]
    pdst = perm[dst0]

    pdeg = np.bincount(pdst, minlength=N)          # per permuted node
    nAf = np.bincount(pdst[psrc < T_LO], minlength=N)
    nBf = np.bincount(pdst[psrc >= T_HI], minlength=N)
    need = np.maximum(np.maximum(nAf, nBf), (pdeg + 1) // 2)

    # block schedule: Dh[b] = max need over all cores' block b
    need_pad = np.zeros(NC * NPAD, dtype=np.int64)
    node_pid = np.arange(N)
    need_pad[(node_pid // NPC) * NPAD + node_pid % NPC] = need
    Dh = need_pad.reshape(NC, NBLK, 128).max(axis=(0, 2)).astype(np.int64)
    Dh = np.maximum(Dh, 1)

    # group blocks with equal Dh, G*Dh <= CHUNK_SLOTS
    groups = []  # (b0, G, D)
    b = 0
    while b < NBLK:
        d = int(Dh[b])
        g = 1
        while (b + g < NBLK and Dh[b + g] == d
               and (g + 1) * d <= max(d, CHUNK_SLOTS)):
            g += 1
        groups.append((b, g, d))
        b += g
    # chunks: consecutive groups, per-region slots <= CHUNK_SLOTS and
    # idx count <= MAX_IDX_PER_GATHER
    chunks = []  # list of (group_lo, group_hi) indices into groups
    offq = np.zeros(NBLK, dtype=np.int64)  # per-block region-slot offset q
    q = 0
    for (b0, g, d) in groups:
        for bb in range(b0, b0 + g):
            offq[bb] = q
            q += d
    SA = int(q)  # per-partition slots per region
    # graded caps: small first chunks so the post-exchange pipeline fills fast
    def cap_for(ci):
        return (32, 64)[ci] if ci < 2 else CHUNK_SLOTS
    lo = 0
    while lo < len(groups):
        hi = lo
        s = 0
        cap = cap_for(len(chunks))
        while hi < len(groups):
            b0, g, d = groups[hi]
            add = g * d
            if s > 0 and (s + add > cap
                          or (s + add) * 128 > MAX_IDX_PER_GATHER):
                break
            s += add
            hi += 1
        chunks.append((lo, hi, s))
        lo = hi
    # split a fat trailing chunk so the exchange isn't gated on one big tail
    if len(chunks) > 1 and chunks[-1][2] > 64 and chunks[-1][1] - chunks[-1][0] > 1:
        glo, ghi, s = chunks.pop()
        mid = glo
        acc = 0
        while mid < ghi - 1 and acc + groups[mid][1] * groups[mid][2] < s // 2:
            acc += groups[mid][1] * groups[mid][2]
            mid += 1
        mid = max(mid, glo + 1)
        s1 = sum(g * d for (_, g, d) in groups[glo:mid])
        if 0 < s1 < s:
            chunks.append((glo, mid, s1))
            chunks.append((mid, ghi, s - s1))
        else:
            chunks.append((glo, ghi, s))

    # per-core slot tables
    # edge assignment: sort edges by (core, pos) then split regions per node
    eorder = np.lexsort((psrc, pdst))
    s_src = psrc[eorder]
    s_dst = pdst[eorder]
    starts = np.searchsorted(s_dst, np.arange(N))
    ends = np.searchsorted(s_dst, np.arange(N) + 1)

    idxA = np.zeros((NC, 128, SA), dtype=np.int16)
    idxB = np.zeros((NC, 128, SA), dtype=np.int16)
    maskA = np.full((NC, 128, SA), -1e30, dtype=np.float32)
    maskB = np.full((NC, 128, SA), -1e30, dtype=np.float32)

    for n in range(N):
        e0, e1 = starts[n], ends[n]
        if e0 == e1:
            continue
        ss = s_src[e0:e1]
        c = n // NPC
        p = n % NPC
        bb = p // 128
        pp = p % 128
        d = int(Dh[bb])
        q0 = int(offq[bb])
        fa = ss[ss < T_LO]
        fb = ss[ss >= T_HI]
        fx = ss[(ss >= T_LO) & (ss < T_HI)]
        na, nb, nd = len(fa), len(fb), len(ss)
        lo_t = max(na, nd - d)
        hi_t = min(na + len(fx), d)
        ta = min(max((nd + 1) // 2, lo_t), hi_t)
        a_list = np.concatenate([fa, fx[: ta - na]])
        b_list = np.concatenate([fb, fx[ta - na:]])
        la, lb = len(a_list), len(b_list)
        assert la <= d and lb <= d, (n, la, lb, d)
        idxA[c, pp, q0:q0 + la] = a_list.astype(np.int16)
        maskA[c, pp, q0:q0 + la] = 0.0
        idxB[c, pp, q0:q0 + lb] = (b_list - T_LO).astype(np.int16)
        maskB[c, pp, q0:q0 + lb] = 0.0

    # wrap idxs for dma_gather: position i = q*128 + p -> [i%16, i//16], x8
    def wrap(idx):  # [128, SA] -> [128, SA*8] int16
        # i = q*128 + p ; element at [i % 16, i // 16]
        flat = idx.transpose(1, 0).reshape(-1)          # i-major
        w16 = flat.reshape(-1, 16).T                    # [16, SA*8]
        return np.tile(w16, (8, 1)).astype(np.int16)

    idxA_w = np.stack([wrap(idxA[c]) for c in range(NC)])
    idxB_w = np.stack([wrap(idxB[c]) for c in range(NC)])
    mask = np.stack([np.concatenate([maskA[c], maskB[c]], axis=1)
                     for c in range(NC)])               # [NC, 128, 2*SA]

    sched = dict(Dh=Dh, groups=groups, chunks=chunks, offq=offq, SA=SA)
    key = hashlib.sha256(
        (str(groups) + str(chunks) + str(L_DEBUG) + os.environ.get("SGAT_DUMP", "")).encode()).hexdigest()[:16]
    return dict(perm=perm, inv_perm=inv_perm, sched=sched, key=key,
                idxA=idxA_w, idxB=idxB_w, mask=mask)


# ----------------------------------------------------------------------------
# weights preprocessing
# ----------------------------------------------------------------------------

def _prep_weights(W0, b0, Ws, att_l, att_r, bs, W16, b16):
    # table_1 = (x @ W0 + b0) @ W1aug ; W1aug = [W1 | W1@al1 | W1@ar1]
    def aug(Wl, al, ar):
        A = np.zeros((H, ROWW), np.float32)
        A[:, :H] = Wl
        A[:, H] = Wl @ al
        A[:, H + 1] = Wl @ ar
        return A

    W1aug = aug(Ws[0], att_l[0], att_r[0])
    wfold = (W0 @ W1aug).astype(np.float32)            # [128, 36]
    bfold = (b0 @ W1aug).astype(np.float32)            # [36]
    waug = np.zeros((L_FULL, H, ROWW), np.float32)
    for l in range(1, L_FULL):
        waug[l - 1] = aug(Ws[l], att_l[l], att_r[l])
    waug[L_FULL - 1, :, :D_OUT] = W16                  # layer-15 tail
    brep = np.tile(bs[:, None, :], (1, 128, 1)).astype(np.float32)
    bfold_rep = np.tile(bfold[None, :], (128, 1)).astype(np.float32)
    b16rep = np.tile(b16[None, :], (128, 1)).astype(np.float32)
    return dict(wfold=wfold, bfold=bfold_rep, waug=waug, brep=brep,
                b16rep=b16rep)


# ----------------------------------------------------------------------------
# program builder
# ----------------------------------------------------------------------------

def _build_program(sched, sim=False):
    groups = sched["groups"]
    chunks = sched["chunks"]
    offq = sched["offq"]
    SA = sched["SA"]
    LN = L_DEBUG

    nc = bacc.Bacc(num_devices=NC, num_swdge_queues=N_QUEUES)
    xT_in = nc.dram_tensor("xT", [D_IN, NPAD], dt.float32, kind="ExternalInput")
    idxA_in = nc.dram_tensor("idxA", [128, SA * 8], dt.int16, kind="ExternalInput")
    idxB_in = nc.dram_tensor("idxB", [128, SA * 8], dt.int16, kind="ExternalInput")
    mask_in = nc.dram_tensor("mask", [128, 2 * SA], dt.float32, kind="ExternalInput")
    wfold_in = nc.dram_tensor("wfold", [D_IN, ROWW], dt.float32, kind="ExternalInput")
    bfold_in = nc.dram_tensor("bfold", [128, ROWW], dt.float32, kind="ExternalInput")
    waug_in = nc.dram_tensor("waug", [L_FULL, H, ROWW], dt.float32, kind="ExternalInput")
    brep_in = nc.dram_tensor("brep", [L_FULL, 128, H], dt.float32, kind="ExternalInput")
    b16_in = nc.dram_tensor("b16rep", [128, D_OUT], dt.float32, kind="ExternalInput")

    qn_state = [0]

    def next_queue():
        q = qn_state[0]
        qn_state[0] = (q + 1) % N_QUEUES
        return q

    DUMP = os.environ.get("SGAT_DUMP", "")
    if DUMP == "table":
        out_d = nc.dram_tensor("out", [N, ROWW], dt.uint16, kind="ExternalOutput")
    elif DUMP == "gbuf":
        csl0 = chunks[0][2]
        out_d = nc.dram_tensor("out", [128, 2 * csl0 * ROWW], dt.uint16,
                               kind="ExternalOutput")
    elif DUMP in ("alpha", "ex", "agg"):
        csl0 = chunks[0][2]
        w = csl0 * H if DUMP == "agg" else csl0
        out_d = nc.dram_tensor("out", [128, 2 * w], dt.float32,
                               kind="ExternalOutput")
    elif LN >= L_FULL:
        out_d = nc.dram_tensor("out", [NPAD, D_OUT], dt.float32, kind="ExternalOutput")
    else:
        out_d = nc.dram_tensor("out", [NPAD, ROWW], dt.uint16, kind="ExternalOutput")

    with tile.TileContext(nc) as tc:
        with tc.tile_pool(name="res", bufs=1) as res, \
             tc.tile_pool(name="gp", bufs=3) as gp, \
             tc.tile_pool(name="wp", bufs=2) as wp, \
             tc.tile_pool(name="sp", bufs=3) as sp, \
             tc.tile_pool(name="tp", bufs=2) as tp, \
             tc.tile_pool(name="pt", bufs=2, space="PSUM") as pt, \
             tc.tile_pool(name="pm", bufs=2, space="PSUM") as pm, \
             tc.tile_pool(name="dram", bufs=2, space="DRAM") as dram:

            nc.gpsimd.load_library(library_config.mlp)

            # residents
            xT = res.tile([D_IN, NPAD], dt.float32)
            nc.sync.dma_start(out=xT[:], in_=xT_in[:])
            idxA = res.tile([128, SA * 8], dt.int16)
            nc.sync.dma_start(out=idxA[:], in_=idxA_in[:])
            idxB = res.tile([128, SA * 8], dt.int16)
            nc.sync.dma_start(out=idxB[:], in_=idxB_in[:])
            maskr = res.tile([128, 2 * SA], dt.float32)
            nc.sync.dma_start(out=maskr[:], in_=mask_in[:])
            wfold = res.tile([D_IN, ROWW], dt.float32)
            nc.sync.dma_start(out=wfold[:], in_=wfold_in[:])
            bfold = res.tile([128, ROWW], dt.float32)
            nc.sync.dma_start(out=bfold[:], in_=bfold_in[:])
            waug = res.tile([H, L_FULL * ROWW], dt.float32)
            nc.sync.dma_start(
                out=waug[:].rearrange("h (l w) -> h l w", l=L_FULL),
                in_=waug_in[:].rearrange("l h w -> h l w"))
            brep = res.tile([128, L_FULL * H], dt.float32)
            nc.sync.dma_start(
                out=brep[:].rearrange("p (l h) -> p l h", l=L_FULL),
                in_=brep_in[:].rearrange("l p h -> p l h"))
            b16r = res.tile([128, D_OUT], dt.float32)
            nc.sync.dma_start(out=b16r[:], in_=b16_in[:])
            ident = res.tile([128, 128], dt.float32)
            make_identity(nc, ident[:])

            own_tabs = [res.tile([128, NBLK, ROWW], dt.bfloat16, name=f"own{i}")
                        for i in range(2)]
            outstage = res.tile([128, NBLK, D_OUT], dt.float32)

            def pack_row(psum_ap, own_tab, b):
                # psum [128, 36] f32 -> own_tab[:, b, :] (hp bf16 + aL/aR f32)
                bf = own_tab[:]
                nc.vector.tensor_copy(out=bf[:, b, 0:H], in_=psum_ap[:, 0:H])
                f32v = own_tab[:].bitcast(dt.float32)
                nc.scalar.copy(out=f32v[:, b, H // 2:H // 2 + 2],
                               in_=psum_ap[:, H:H + 2])

            # ---------------- conv0 + fold into table_1 -----------------
            own = own_tabs[0]
            for b in range(NBLK):
                ps = pm.tile([128, ROWW], dt.float32, space="PSUM", tag="mm")
                nc.tensor.matmul(out=ps[:], lhsT=xT[:, b * 128:(b + 1) * 128],
                                 rhs=wfold[:], start=True, stop=True)
                ps2 = sp.tile([128, ROWW], dt.float32, tag="c0add")
                nc.vector.tensor_tensor(out=ps2[:], in0=ps[:], in1=bfold[:],
                                        op=mybir.AluOpType.add)
                pack_row(ps2[:], own, b)

            def exchange(own_tab, li):
                bounce = dram.tile([NPAD, ROWW], dt.bfloat16, tag="bounce")
                bv = bounce[:].rearrange("(b p) w -> p b w", p=128)
                bsplit = [0, 12, 24, 36, NBLK]
                for si in range(4):
                    s0, s1 = bsplit[si], bsplit[si + 1]
                    nc.sync.dma_start(out=bv[:, s0:s1],
                                      in_=own_tab[:, s0:s1])
                table = dram.tile([N, TABW], dt.bfloat16, tag="table")
                if sim:
                    # timing-equivalent stand-in for AllGather + spread
                    for c in range(NC):
                        nc.sync.dma_start(
                            out=table[c * NPC:(c + 1) * NPC, 0:ROWW],
                            in_=bounce[0:NPC, :])
                    return table
                agout = dram.tile([N, ROWW], dt.bfloat16, tag="agout")
                nc.gpsimd.collective_compute(
                    "AllGather", mybir.AluOpType.bypass,
                    replica_groups=[list(range(NC))],
                    ins=[bounce[0:NPC, :]], outs=[agout[:]])
                nc.sync.dma_start(out=table[:, 0:ROWW], in_=agout[:])
                return table

            if LN == 0 and not DUMP:
                nc.sync.dma_start(
                    out=out_d[:].rearrange("(b p) w -> p b w", p=128),
                    in_=own[:].bitcast(dt.uint16))
            table = exchange(own, 0)
            if DUMP == "table":
                nc.sync.dma_start(out=out_d[:],
                                  in_=table[:, 0:ROWW].bitcast(dt.uint16))
            elif DUMP == "gbuf":
                glo, ghi, csl = chunks[0]
                q0 = int(offq[groups[glo][0]])
                gb = gp.tile([128, 2, csl, ROWW], dt.bfloat16, tag="gb")
                for r in range(2):
                    tab_view = table[0:T_HI, 0:ROWW] if r == 0 \
                        else table[T_LO:N, 0:ROWW]
                    idxr = idxA if r == 0 else idxB
                    nidx = csl * 128
                    nc.gpsimd.dma_gather(
                        out_ap=gb[:, r, :, :], in_ap=tab_view,
                        idxs_ap=idxr[:, q0 * 8:(q0 + csl) * 8],
                        num_idxs=nidx, num_idxs_reg=nidx,
                        elem_size=ROWW, elem_step=TABW, single_packet=False,
                        queue_num=next_queue())
                nc.sync.dma_start(
                    out=out_d[:],
                    in_=gb[:].rearrange("p r q w -> p (r q w)").bitcast(dt.uint16))
            if DUMP in ("table", "gbuf"):
                LN_eff = 0
            elif DUMP:
                LN_eff = 1
            else:
                LN_eff = LN
            dbg = None
            if DUMP in ("alpha", "ex", "agg"):
                _w = chunks[0][2] * (H if DUMP == "agg" else 1)
                dbg = res.tile([128, 2 * _w], dt.float32, name="dbg")

            # ---------------- layers ----------------
            for li in range(1, LN_eff + 1):
                own_prev = own_tabs[(li + 1) % 2]
                own_new = own_tabs[li % 2]
                last = (li == L_FULL)
                for (glo, ghi, csl) in chunks:
                    b0 = groups[glo][0]
                    q0 = int(offq[b0])
                    gb = gp.tile([128, 2, csl, ROWW], dt.bfloat16, tag="gb")
                    for r in range(2):
                        tab_view = table[0:T_HI, 0:ROWW] if r == 0 \
                            else table[T_LO:N, 0:ROWW]
                        idxr = idxA if r == 0 else idxB
                        nidx = csl * 128
                        nc.gpsimd.dma_gather(
                            out_ap=gb[:, r, :, :],
                            in_ap=tab_view,
                            idxs_ap=idxr[:, q0 * 8:(q0 + csl) * 8],
                            num_idxs=nidx, num_idxs_reg=nidx,
                            elem_size=ROWW, elem_step=TABW,
                            single_packet=False, queue_num=next_queue())
                    for gi in range(glo, ghi):
                        bg, G, D = groups[gi]
                        qa = int(offq[bg]) - q0
                        GD = G * D
                        S2 = 2 * GD
                        # views
                        hp_g = gb[:, :, qa:qa + GD, 0:H].rearrange(
                            "p r (g d) f -> p r g d f", g=G)
                        gf32 = gb[:].bitcast(dt.float32)
                        aL_g = gf32[:, :, qa:qa + GD, H // 2]       # [p,2,GD]
                        ownf = own_prev[:].bitcast(dt.float32)
                        aR_o = ownf[:, bg:bg + G, H // 2 + 1]       # [p,G]
                        hp_o = own_prev[:, bg:bg + G, 0:H]          # [p,G,32]

                        prod = wp.tile([128, S2, H], dt.bfloat16, tag="prod")
                        prodv = prod[:].rearrange("p (r q) f -> p r q f", r=2)
                        for r in range(2):
                            nc.vector.tensor_tensor(
                                out=prodv[:, r].rearrange(
                                    "p (g d) f -> p g d f", g=G),
                                in0=gb[:, r, qa:qa + GD, 0:H].rearrange(
                                    "p (g d) f -> p g d f", g=G),
                                in1=hp_o.unsqueeze(2)
                                .broadcast_to([128, G, D, H]),
                                op=mybir.AluOpType.mult)
                        logit = sp.tile([128, S2], dt.float32, tag="logit")
                        nc.vector.tensor_reduce(
                            out=logit[:], in_=prod[:],
                            axis=mybir.AxisListType.X, op=mybir.AluOpType.add,
                            negate=True)
                        sig = sp.tile([128, S2], dt.float32, tag="sig")
                        nc.scalar.activation(
                            out=sig[:], in_=logit[:],
                            func=mybir.ActivationFunctionType.Exp)
                        nc.vector.tensor_scalar(
                            out=sig[:], in0=sig[:], scalar1=1.0, scalar2=None,
                            op0=mybir.AluOpType.add)
                        nc.vector.reciprocal(out=sig[:], in_=sig[:])
                        alpha = sp.tile([128, S2], dt.float32, tag="alpha")
                        nc.vector.tensor_tensor(
                            out=alpha[:].rearrange("p (r g d) -> p r g d",
                                                   r=2, g=G),
                            in0=aL_g.rearrange("p r (g d) -> p r g d", g=G),
                            in1=aR_o.unsqueeze(1).unsqueeze(3).broadcast_to(
                                [128, 2, G, D]),
                            op=mybir.AluOpType.add)
                        nc.vector.tensor_tensor(out=alpha[:], in0=alpha[:],
                                                in1=sig[:],
                                                op=mybir.AluOpType.mult)
                        asc = sp.tile([128, S2], dt.float32, tag="asc")
                        nc.vector.tensor_scalar(
                            out=asc[:], in0=alpha[:], scalar1=NEG, scalar2=None,
                            op0=mybir.AluOpType.mult)
                        nc.vector.tensor_tensor(
                            out=alpha[:], in0=alpha[:], in1=asc[:],
                            op=mybir.AluOpType.max)
                        mk = maskr[:].rearrange("p (r q) -> p r q", r=2)[
                            :, :, qa + q0:qa + q0 + GD]
                        nc.vector.tensor_tensor(
                            out=alpha[:].rearrange("p (r q) -> p r q", r=2),
                            in0=alpha[:].rearrange("p (r q) -> p r q", r=2),
                            in1=mk, op=mybir.AluOpType.add)
                        if dbg is not None and li == 1 and glo == 0 and DUMP == "alpha":
                            nc.vector.tensor_copy(
                                out=dbg[:].rearrange("p (r q) -> p r q", r=2)[
                                    :, :, qa:qa + GD],
                                in_=alpha[:].rearrange("p (r q) -> p r q", r=2))
                        am2 = sp.tile([128, 2 * G], dt.float32, tag="am2")
                        nc.vector.tensor_reduce(
                            out=am2[:],
                            in_=alpha[:].rearrange("p (rg d) -> p rg d", d=D),
                            axis=mybir.AxisListType.X, op=mybir.AluOpType.max)
                        nam = sp.tile([128, G], dt.float32, tag="nam")
                        nc.vector.tensor_reduce(
                            out=nam[:],
                            in_=am2[:].rearrange("p (r g) -> p g r", r=2),
                            axis=mybir.AxisListType.X, op=mybir.AluOpType.max,
                            negate=True)
                        nc.vector.tensor_tensor(
                            out=alpha[:].rearrange("p (r g d) -> p r g d",
                                                   r=2, g=G),
                            in0=alpha[:].rearrange("p (r g d) -> p r g d",
                                                   r=2, g=G),
                            in1=nam[:].unsqueeze(1).unsqueeze(3).broadcast_to(
                                [128, 2, G, D]),
                            op=mybir.AluOpType.add)
                        ex = sp.tile([128, S2], dt.bfloat16, tag="ex")
                        nc.scalar.activation(
                            out=ex[:], in_=alpha[:],
                            func=mybir.ActivationFunctionType.Exp)
                        if dbg is not None and li == 1 and glo == 0 and DUMP == "ex":
                            nc.vector.tensor_copy(
                                out=dbg[:].rearrange("p (r q) -> p r q", r=2)[
                                    :, :, qa:qa + GD],
                                in_=ex[:].rearrange("p (r q) -> p r q", r=2))
                        den2 = sp.tile([128, 2 * G], dt.float32, tag="den2")
                        nc.vector.tensor_reduce(
                            out=den2[:],
                            in_=ex[:].rearrange("p (rg d) -> p rg d", d=D),
                            axis=mybir.AxisListType.X, op=mybir.AluOpType.add)
                        rden = sp.tile([128, G], dt.float32, tag="rden")
                        den1 = sp.tile([128, G], dt.float32, tag="den1")
                        nc.vector.tensor_reduce(
                            out=den1[:],
                            in_=den2[:].rearrange("p (r g) -> p g r", r=2),
                            axis=mybir.AxisListType.X, op=mybir.AluOpType.add)
                        nc.vector.reciprocal(out=rden[:], in_=den1[:])
                        wv = wp.tile([128, S2, H], dt.bfloat16, tag="wv")
                        nc.vector.tensor_tensor(
                            out=wv[:].rearrange("p (r q) f -> p r q f", r=2),
                            in0=gb[:, :, qa:qa + GD, 0:H],
                            in1=ex[:].rearrange("p (r q) -> p r q", r=2)
                            .unsqueeze(3).broadcast_to([128, 2, GD, H]),
                            op=mybir.AluOpType.mult)
                        agg2 = tp.tile([128, 2, G, H], dt.float32, tag="agg2")
                        nc.vector.tensor_reduce(
                            out=agg2[:].rearrange("p r g f -> p (r g) f"),
                            in_=wv[:].rearrange(
                                "p (r g d) f -> p (r g) f d", r=2, g=G),
                            axis=mybir.AxisListType.X,
                            op=mybir.AluOpType.add)
                        agg = tp.tile([128, G, H], dt.float32, tag="agg")
                        nc.vector.tensor_tensor(
                            out=agg[:], in0=agg2[:, 0], in1=agg2[:, 1],
                            op=mybir.AluOpType.add)
                        nc.vector.tensor_tensor(
                            out=agg[:], in0=agg[:],
                            in1=rden[:].unsqueeze(2).broadcast_to([128, G, H]),
                            op=mybir.AluOpType.mult)
                        if dbg is not None and li == 1 and glo == 0 and DUMP == "agg":
                            nc.vector.tensor_copy(
                                out=dbg[:, qa * H:(qa + G) * H],
                                in_=agg[:].rearrange("p g h -> p (g h)"))
                        nc.vector.tensor_tensor(
                            out=agg[:], in0=agg[:],
                            in1=brep[:].rearrange("p (l h) -> p l h",
                                                  l=L_FULL)[:, li - 1]
                            .unsqueeze(1).broadcast_to([128, G, H]),
                            op=mybir.AluOpType.add)
                        hnext = tp.tile([128, G, H], dt.float32, tag="hnext")
                        nc.vector.tensor_scalar(
                            out=hnext[:], in0=agg[:], scalar1=0.0, scalar2=None,
                            op0=mybir.AluOpType.max)
                        # tails per block
                        wslice = waug[:].rearrange(
                            "h (l w) -> h l w", l=L_FULL)[:, li - 1, :]
                        for gg in range(G):
                            b = bg + gg
                            hT_ps = pt.tile([H, 128], dt.float32,
                                            space="PSUM", tag="hT")
                            nc.tensor.transpose(out=hT_ps[:],
                                                in_=hnext[:, gg, :],
                                                identity=ident[:])
                            hT = sp.tile([H, 128], dt.float32, tag="hTs")
                            nc.scalar.copy(out=hT[:], in_=hT_ps[:])
                            mm = pm.tile([128, ROWW], dt.float32,
                                         space="PSUM", tag="mm")
                            if last:
                                nc.tensor.matmul(out=mm[:, 0:D_OUT],
                                                 lhsT=hT[:],
                                                 rhs=wslice[:, 0:D_OUT],
                                                 start=True, stop=True)
                                nc.vector.tensor_tensor(
                                    out=outstage[:, b, :],
                                    in0=mm[:, 0:D_OUT], in1=b16r[:],
                                    op=mybir.AluOpType.add)
                            else:
                                nc.tensor.matmul(out=mm[:], lhsT=hT[:],
                                                 rhs=wslice[:],
                                                 start=True, stop=True)
                                pack_row(mm[:], own_new, b)
                if dbg is not None and li == 1:
                    nc.sync.dma_start(out=out_d[:], in_=dbg[:])
                    break
                if last:
                    nc.sync.dma_start(
                        out=out_d[:].rearrange("(b p) w -> p b w", p=128),
                        in_=outstage[:])
                elif li == LN:
                    nc.sync.dma_start(
                        out=out_d[:].rearrange("(b p) w -> p b w", p=128),
                        in_=own_new[:].bitcast(dt.uint16))
                else:
                    table = exchange(own_new, li)

    nc.compile()
    return nc


# ----------------------------------------------------------------------------
# entry point
# ----------------------------------------------------------------------------

_CACHE = {}
LAST_RESULT = None


def kernel(x, edge_index, W0, b0, Ws, att_l, att_r, bs, W16, b16):
    global LAST_RESULT
    x = np.asarray(x, dtype=np.float32)
    edge_index = np.asarray(edge_index)
    pre = _preprocess(edge_index)
    wts = _prep_weights(np.asarray(W0, np.float32), np.asarray(b0, np.float32),
                        np.asarray(Ws, np.float32),
                        np.asarray(att_l, np.float32),
                        np.asarray(att_r, np.float32),
                        np.asarray(bs, np.float32),
                        np.asarray(W16, np.float32),
                        np.asarray(b16, np.float32))
    key = pre["key"]
    if key not in _CACHE:
        _CACHE[key] = _build_program(pre["sched"])
    nc = _CACHE[key]

    inv_perm = pre["inv_perm"]
    in_maps = []
    for c in range(NC):
        pids = np.arange(c * NPC, (c + 1) * NPC)
        orig = inv_perm[pids]
        xT = np.zeros((D_IN, NPAD), np.float32)
        xT[:, 0:NPC] = x[orig].T
        in_maps.append(dict(
            xT=xT, idxA=pre["idxA"][c], idxB=pre["idxB"][c],
            mask=pre["mask"][c].reshape(128, -1),
            wfold=wts["wfold"], bfold=wts["bfold"], waug=wts["waug"],
            brep=wts["brep"], b16rep=wts["b16rep"]))

    res = bass_utils.run_bass_kernel_spmd(
        nc, in_maps, core_ids=list(range(NC)),
        tmpdir=os.environ.get("SGAT_TMPDIR") or None)
    LAST_RESULT = res

    if L_DEBUG >= L_FULL:
        out = np.zeros((N, D_OUT), np.float32)
        for c in range(NC):
            pids = np.arange(c * NPC, (c + 1) * NPC)
            out[inv_perm[pids]] = res.results[c]["out"][0:NPC]
        return out
    else:
        # debug: return raw table_{L+1} rows per permuted id
        out = np.zeros((N, ROWW), np.uint16)
        for c in range(NC):
            pids = np.arange(c * NPC, (c + 1) * NPC)
            out[inv_perm[pids]] = res.results[c]["out"][0:NPC]
        return out



# revision 24
# speedup vs baseline: 1.2148x; 1.0966x over previous
"""SuperGAT x15 Trainium2 kernel (8 NeuronCores, SPMD).

Self-contained: hardcodes all shapes. Strategy:
- Nodes permuted by "need" (balanced split degree), striped across 8 cores
  (core = rank % 8, pos = rank // 8). Each core owns 6250 nodes and all
  edges whose dst it owns.
- Per layer, each core holds a replicated DRAM table of rows
  [hp(32) bf16 | aL f32 | aR f32] = 36 bf16-slots = 72B at 256B stride.
- Messages gathered per edge-slot via dma_gather (int16 idxs). The int16
  range limit (32767) is handled with two overlapping table views:
  region A = rows [0, 32768), region B = rows [17232, 50000).
  Each node's in-edges are split between regions (balanced), padded to a
  per-block schedule Dh[b] shared by all cores (SPMD: one program).
- Layout C: node-per-partition, slots along free axis. Segment softmax =
  free-axis reductions. No per-edge scatter: aggregation output lands
  per-node directly.
- Per-layer exchange: own table rows -> DRAM bounce -> AllGather ->
  spread DMA into the 256B-stride gather table.
"""
import os
import hashlib
import numpy as np
import ml_dtypes

import concourse.bacc as bacc
import concourse.bass as bass
import concourse.tile as tile
from concourse import mybir, bass_utils, library_config
from concourse.masks import make_identity

dt = mybir.dt

# problem constants
N = 50000
E = 800000
D_IN = 128
H = 32
D_OUT = 16
L_FULL = 15
NEG = 0.2
NC = 8
NPC = N // NC            # 6250 nodes per core
NBLK = (NPC + 127) // 128  # 49 blocks
NPAD = NBLK * 128        # 6272 padded positions
T_HI = 32768
T_LO = N - T_HI          # 17232
ROWW = 36                # bf16 slots per table row (72B payload)
TABW = 128               # bf16 slots per table row stride (256B)

L_DEBUG = int(os.environ.get("SGAT_LAYERS", str(L_FULL)))
MAX_IDX_PER_GATHER = 16000
CHUNK_SLOTS = int(os.environ.get("SGAT_CHUNK", "120"))  # per-partition per-region
N_QUEUES = int(os.environ.get("SGAT_QUEUES", "4"))      # SWDGE rings to cycle


def _patch_dma_gather_assert():
    import inspect, textwrap
    if getattr(bass.BassGpSimd.dma_gather, "_sgat_patched", False):
        return
    src = inspect.getsource(bass.BassGpSimd.dma_gather)
    src = src.replace(
        "assert (\n            elem_size_bytes > 0 and elem_size_bytes % 256 == 0\n        )  # transpose restriction",
        "assert elem_size_bytes > 0")
    src = textwrap.dedent(src)
    ns = dict(bass.BassGpSimd.dma_gather.__globals__)
    exec(src, ns)
    fn = ns["dma_gather"]
    fn._sgat_patched = True
    bass.BassGpSimd.dma_gather = fn


_patch_dma_gather_assert()


# ----------------------------------------------------------------------------
# host-side graph preprocessing
# ----------------------------------------------------------------------------

def _preprocess(edge_index):
    src0 = edge_index[0].astype(np.int64)
    dst0 = edge_index[1].astype(np.int64)
    loops = np.arange(N, dtype=np.int64)
    src0 = np.concatenate([src0, loops])
    dst0 = np.concatenate([dst0, loops])

    deg = np.bincount(dst0, minlength=N)
    # Permutation sorted by degree (need ~ ceil(deg/2) + split imbalance;
    # the schedule below uses the ACTUAL per-node need, so correctness does
    # not depend on this ordering -- only padding efficiency does).
    rank_of = np.argsort(-deg, kind="stable")      # rank -> orig node
    # Band-swap: within each 8-rank stratum (one node per core, same blocks),
    # send the two highest OUT-degree nodes to cores 3 and 4 -- their whole
    # PID range lies inside the int16 overlap band [T_LO, T_HI), so more
    # edges become region-flexible and the padded schedule shrinks.
    odeg = np.bincount(src0, minlength=N)          # out-degree (incl loop)
    ro = rank_of[: (N // NC) * NC].reshape(-1, NC)  # [stratum, 8 nodes]
    od = odeg[ro]
    order = np.argsort(-od, axis=1, kind="stable")  # per-stratum by out-deg
    # core slots ordered by in-band preference: 3,4 fully in band; 2,5 partly
    slot_pref = np.array([3, 4, 2, 5, 1, 6, 0, 7])
    new_ro = np.empty_like(ro)
    new_ro[np.arange(len(ro))[:, None], slot_pref[None, :]] = np.take_along_axis(
        ro, order, axis=1)
    rank_of = rank_of.copy()
    rank_of[: len(ro) * NC] = new_ro.reshape(-1)
    r = np.arange(N, dtype=np.int64)
    pid_of_rank = (r % NC) * NPC + r // NC
    perm = np.empty(N, dtype=np.int64)             # orig -> permuted id
    perm[rank_of] = pid_of_rank
    inv_perm = np.empty(N, dtype=np.int64)         # permuted id -> orig
    inv_perm[perm] = np.arange(N, dtype=np.int64)

    psrc = perm[src0]
    pdst = perm[dst0]

    pdeg = np.bincount(pdst, minlength=N)          # per permuted node
    nAf = np.bincount(pdst[psrc < T_LO], minlength=N)
    nBf = np.bincount(pdst[psrc >= T_HI], minlength=N)
    need = np.maximum(np.maximum(nAf, nBf), (pdeg + 1) // 2)

    # block schedule: Dh[b] = max need over all cores' block b
    need_pad = np.zeros(NC * NPAD, dtype=np.int64)
    node_pid = np.arange(N)
    need_pad[(node_pid // NPC) * NPAD + node_pid % NPC] = need
    Dh = need_pad.reshape(NC, NBLK, 128).max(axis=(0, 2)).astype(np.int64)
    Dh = np.maximum(Dh, 1)

    # group blocks with equal Dh, G*Dh <= CHUNK_SLOTS
    groups = []  # (b0, G, D)
    b = 0
    while b < NBLK:
        d = int(Dh[b])
        g = 1
        while (b + g < NBLK and Dh[b + g] == d
               and (g + 1) * d <= max(d, CHUNK_SLOTS)):
            g += 1
        groups.append((b, g, d))
        b += g
    # chunks: consecutive groups, per-region slots <= CHUNK_SLOTS and
    # idx count <= MAX_IDX_PER_GATHER
    chunks = []  # list of (group_lo, group_hi) indices into groups
    offq = np.zeros(NBLK, dtype=np.int64)  # per-block region-slot offset q
    q = 0
    for (b0, g, d) in groups:
        for bb in range(b0, b0 + g):
            offq[bb] = q
            q += d
    SA = int(q)  # per-partition slots per region
    # graded caps: small first chunks so the post-exchange pipeline fills fast
    def cap_for(ci):
        return (32, 64)[ci] if ci < 2 else CHUNK_SLOTS
    lo = 0
    while lo < len(groups):
        hi = lo
        s = 0
        cap = cap_for(len(chunks))
        while hi < len(groups):
            b0, g, d = groups[hi]
            add = g * d
            if s > 0 and (s + add > cap
                          or (s + add) * 128 > MAX_IDX_PER_GATHER):
                break
            s += add
            hi += 1
        chunks.append((lo, hi, s))
        lo = hi
    # split a fat trailing chunk so the exchange isn't gated on one big tail
    if len(chunks) > 1 and chunks[-1][2] > 64 and chunks[-1][1] - chunks[-1][0] > 1:
        glo, ghi, s = chunks.pop()
        mid = glo
        acc = 0
        while mid < ghi - 1 and acc + groups[mid][1] * groups[mid][2] < s // 2:
            acc += groups[mid][1] * groups[mid][2]
            mid += 1
        mid = max(mid, glo + 1)
        s1 = sum(g * d for (_, g, d) in groups[glo:mid])
        if 0 < s1 < s:
            chunks.append((glo, mid, s1))
            chunks.append((mid, ghi, s - s1))
        else:
            chunks.append((glo, ghi, s))

    # per-core slot tables
    # edge assignment: sort edges by (core, pos) then split regions per node
    eorder = np.lexsort((psrc, pdst))
    s_src = psrc[eorder]
    s_dst = pdst[eorder]
    starts = np.searchsorted(s_dst, np.arange(N))
    ends = np.searchsorted(s_dst, np.arange(N) + 1)

    idxA = np.zeros((NC, 128, SA), dtype=np.int16)
    idxB = np.zeros((NC, 128, SA), dtype=np.int16)
    maskA = np.full((NC, 128, SA), -1e30, dtype=np.float32)
    maskB = np.full((NC, 128, SA), -1e30, dtype=np.float32)

    for n in range(N):
        e0, e1 = starts[n], ends[n]
        if e0 == e1:
            continue
        ss = s_src[e0:e1]
        c = n // NPC
        p = n % NPC
        bb = p // 128
        pp = p % 128
        d = int(Dh[bb])
        q0 = int(offq[bb])
        fa = ss[ss < T_LO]
        fb = ss[ss >= T_HI]
        fx = ss[(ss >= T_LO) & (ss < T_HI)]
        na, nb, nd = len(fa), len(fb), len(ss)
        lo_t = max(na, nd - d)
        hi_t = min(na + len(fx), d)
        ta = min(max((nd + 1) // 2, lo_t), hi_t)
        a_list = np.concatenate([fa, fx[: ta - na]])
        b_list = np.concatenate([fb, fx[ta - na:]])
        la, lb = len(a_list), len(b_list)
        assert la <= d and lb <= d, (n, la, lb, d)
        idxA[c, pp, q0:q0 + la] = a_list.astype(np.int16)
        maskA[c, pp, q0:q0 + la] = 0.0
        idxB[c, pp, q0:q0 + lb] = (b_list - T_LO).astype(np.int16)
        maskB[c, pp, q0:q0 + lb] = 0.0

    # wrap idxs for dma_gather: position i = q*128 + p -> [i%16, i//16], x8
    def wrap(idx):  # [128, SA] -> [128, SA*8] int16
        # i = q*128 + p ; element at [i % 16, i // 16]
        flat = idx.transpose(1, 0).reshape(-1)          # i-major
        w16 = flat.reshape(-1, 16).T                    # [16, SA*8]
        return np.tile(w16, (8, 1)).astype(np.int16)

    idxA_w = np.stack([wrap(idxA[c]) for c in range(NC)])
    idxB_w = np.stack([wrap(idxB[c]) for c in range(NC)])
    mask = np.stack([np.concatenate([maskA[c], maskB[c]], axis=1)
                     for c in range(NC)])               # [NC, 128, 2*SA]

    sched = dict(Dh=Dh, groups=groups, chunks=chunks, offq=offq, SA=SA)
    key = hashlib.sha256(
        (str(groups) + str(chunks) + str(L_DEBUG) + os.environ.get("SGAT_DUMP", "")).encode()).hexdigest()[:16]
    return dict(perm=perm, inv_perm=inv_perm, sched=sched, key=key,
                idxA=idxA_w, idxB=idxB_w, mask=mask)


# ----------------------------------------------------------------------------
# weights preprocessing
# ----------------------------------------------------------------------------

def _prep_weights(W0, b0, Ws, att_l, att_r, bs, W16, b16):
    # table_1 = (x @ W0 + b0) @ W1aug ; W1aug = [W1 | W1@al1 | W1@ar1]
    def aug(Wl, al, ar):
        A = np.zeros((H, ROWW), np.float32)
        A[:, :H] = Wl
        A[:, H] = Wl @ al
        A[:, H + 1] = Wl @ ar
        return A

    W1aug = aug(Ws[0], att_l[0], att_r[0])
    wfold = (W0 @ W1aug).astype(np.float32)            # [128, 36]
    bfold = (b0 @ W1aug).astype(np.float32)            # [36]
    waug = np.zeros((L_FULL, H, ROWW), np.float32)
    for l in range(1, L_FULL):
        waug[l - 1] = aug(Ws[l], att_l[l], att_r[l])
    waug[L_FULL - 1, :, :D_OUT] = W16                  # layer-15 tail
    brep = np.tile(bs[:, None, :], (1, 128, 1)).astype(np.float32)
    bfold_rep = np.tile(bfold[None, :], (128, 1)).astype(np.float32)
    b16rep = np.tile(b16[None, :], (128, 1)).astype(np.float32)
    return dict(wfold=wfold, bfold=bfold_rep, waug=waug, brep=brep,
                b16rep=b16rep)


# ----------------------------------------------------------------------------
# program builder
# ----------------------------------------------------------------------------

def _build_program(sched, sim=False):
    groups = sched["groups"]
    chunks = sched["chunks"]
    offq = sched["offq"]
    SA = sched["SA"]
    LN = L_DEBUG

    nc = bacc.Bacc(num_devices=NC, num_swdge_queues=N_QUEUES)
    xT_in = nc.dram_tensor("xT", [D_IN, NPAD], dt.float32, kind="ExternalInput")
    idxA_in = nc.dram_tensor("idxA", [128, SA * 8], dt.int16, kind="ExternalInput")
    idxB_in = nc.dram_tensor("idxB", [128, SA * 8], dt.int16, kind="ExternalInput")
    mask_in = nc.dram_tensor("mask", [128, 2 * SA], dt.float32, kind="ExternalInput")
    wfold_in = nc.dram_tensor("wfold", [D_IN, ROWW], dt.float32, kind="ExternalInput")
    bfold_in = nc.dram_tensor("bfold", [128, ROWW], dt.float32, kind="ExternalInput")
    waug_in = nc.dram_tensor("waug", [L_FULL, H, ROWW], dt.float32, kind="ExternalInput")
    brep_in = nc.dram_tensor("brep", [L_FULL, 128, H], dt.float32, kind="ExternalInput")
    b16_in = nc.dram_tensor("b16rep", [128, D_OUT], dt.float32, kind="ExternalInput")

    qn_state = [0]

    def next_queue():
        q = qn_state[0]
        qn_state[0] = (q + 1) % N_QUEUES
        return q

    DUMP = os.environ.get("SGAT_DUMP", "")
    if DUMP == "table":
        out_d = nc.dram_tensor("out", [N, ROWW], dt.uint16, kind="ExternalOutput")
    elif DUMP == "gbuf":
        csl0 = chunks[0][2]
        out_d = nc.dram_tensor("out", [128, 2 * csl0 * ROWW], dt.uint16,
                               kind="ExternalOutput")
    elif DUMP in ("alpha", "ex", "agg"):
        csl0 = chunks[0][2]
        w = csl0 * H if DUMP == "agg" else csl0
        out_d = nc.dram_tensor("out", [128, 2 * w], dt.float32,
                               kind="ExternalOutput")
    elif LN >= L_FULL:
        out_d = nc.dram_tensor("out", [NPAD, D_OUT], dt.float32, kind="ExternalOutput")
    else:
        out_d = nc.dram_tensor("out", [NPAD, ROWW], dt.uint16, kind="ExternalOutput")

    with tile.TileContext(nc) as tc:
        with tc.tile_pool(name="res", bufs=1) as res, \
             tc.tile_pool(name="gp", bufs=3) as gp, \
             tc.tile_pool(name="wp", bufs=2) as wp, \
             tc.tile_pool(name="sp", bufs=3) as sp, \
             tc.tile_pool(name="tp", bufs=2) as tp, \
             tc.tile_pool(name="pt", bufs=2, space="PSUM") as pt, \
             tc.tile_pool(name="pm", bufs=2, space="PSUM") as pm, \
             tc.tile_pool(name="dram", bufs=2, space="DRAM") as dram:

            nc.gpsimd.load_library(library_config.mlp)

            # residents
            xT = res.tile([D_IN, NPAD], dt.float32)
            nc.sync.dma_start(out=xT[:], in_=xT_in[:])
            idxA = res.tile([128, SA * 8], dt.int16)
            nc.sync.dma_start(out=idxA[:], in_=idxA_in[:])
            idxB = res.tile([128, SA * 8], dt.int16)
            nc.sync.dma_start(out=idxB[:], in_=idxB_in[:])
            maskr = res.tile([128, 2 * SA], dt.float32)
            nc.sync.dma_start(out=maskr[:], in_=mask_in[:])
            wfold = res.tile([D_IN, ROWW], dt.float32)
            nc.sync.dma_start(out=wfold[:], in_=wfold_in[:])
            bfold = res.tile([128, ROWW], dt.float32)
            nc.sync.dma_start(out=bfold[:], in_=bfold_in[:])
            waug = res.tile([H, L_FULL * ROWW], dt.float32)
            nc.sync.dma_start(
                out=waug[:].rearrange("h (l w) -> h l w", l=L_FULL),
                in_=waug_in[:].rearrange("l h w -> h l w"))
            brep = res.tile([128, L_FULL * H], dt.float32)
            nc.sync.dma_start(
                out=brep[:].rearrange("p (l h) -> p l h", l=L_FULL),
                in_=brep_in[:].rearrange("l p h -> p l h"))
            b16r = res.tile([128, D_OUT], dt.float32)
            nc.sync.dma_start(out=b16r[:], in_=b16_in[:])
            ident = res.tile([128, 128], dt.float32)
            make_identity(nc, ident[:])

            own_tabs = [res.tile([128, NBLK, ROWW], dt.bfloat16, name=f"own{i}")
                        for i in range(2)]
            outstage = res.tile([128, NBLK, D_OUT], dt.float32)

            def pack_row(psum_ap, own_tab, b):
                # psum [128, 36] f32 -> own_tab[:, b, :] (hp bf16 + aL/aR f32)
                bf = own_tab[:]
                nc.vector.tensor_copy(out=bf[:, b, 0:H], in_=psum_ap[:, 0:H])
                f32v = own_tab[:].bitcast(dt.float32)
                nc.scalar.copy(out=f32v[:, b, H // 2:H // 2 + 2],
                               in_=psum_ap[:, H:H + 2])

            # ---------------- conv0 + fold into table_1 -----------------
            own = own_tabs[0]
            for b in range(NBLK):
                ps = pm.tile([128, ROWW], dt.float32, space="PSUM", tag="mm")
                nc.tensor.matmul(out=ps[:], lhsT=xT[:, b * 128:(b + 1) * 128],
                                 rhs=wfold[:], start=True, stop=True)
                ps2 = sp.tile([128, ROWW], dt.float32, tag="c0add")
                nc.vector.tensor_tensor(out=ps2[:], in0=ps[:], in1=bfold[:],
                                        op=mybir.AluOpType.add)
                pack_row(ps2[:], own, b)

            def exchange(own_tab, li):
                bounce = dram.tile([NPAD, ROWW], dt.bfloat16, tag="bounce")
                bv = bounce[:].rearrange("(b p) w -> p b w", p=128)
                bsplit = [0, 12, 24, 36, NBLK]
                for si in range(4):
                    s0, s1 = bsplit[si], bsplit[si + 1]
                    nc.sync.dma_start(out=bv[:, s0:s1],
                                      in_=own_tab[:, s0:s1])
                table = dram.tile([N, TABW], dt.bfloat16, tag="table")
                if sim:
                    # timing-equivalent stand-in for AllGather + spread
                    for c in range(NC):
                        nc.sync.dma_start(
                            out=table[c * NPC:(c + 1) * NPC, 0:ROWW],
                            in_=bounce[0:NPC, :])
                    return table
                # split AG: first 24 blocks rendezvous while trailing chunks
                # still compute; only the smaller tail AG stays serial
                agout = dram.tile([N, ROWW], dt.bfloat16, tag="agout")
                H0 = 24 * 128
                rg = [list(range(NC))]
                nc.gpsimd.collective_compute(
                    "AllGather", mybir.AluOpType.bypass, replica_groups=rg,
                    ins=[bounce[0:H0, :]], outs=[agout[0:NC * H0, :]])
                nc.gpsimd.collective_compute(
                    "AllGather", mybir.AluOpType.bypass, replica_groups=rg,
                    ins=[bounce[H0:NPC, :]], outs=[agout[NC * H0:NC * NPC, :]])
                tv = table[:].rearrange("(c q) w -> c q w", c=NC)
                nc.sync.dma_start(
                    out=tv[:, 0:H0, 0:ROWW],
                    in_=agout[0:NC * H0, :].rearrange("(c q) w -> c q w", c=NC))
                nc.sync.dma_start(
                    out=tv[:, H0:NPC, 0:ROWW],
                    in_=agout[NC * H0:NC * NPC, :].rearrange(
                        "(c q) w -> c q w", c=NC))
                return table

            if LN == 0 and not DUMP:
                nc.sync.dma_start(
                    out=out_d[:].rearrange("(b p) w -> p b w", p=128),
                    in_=own[:].bitcast(dt.uint16))
            table = exchange(own, 0)
            if DUMP == "table":
                nc.sync.dma_start(out=out_d[:],
                                  in_=table[:, 0:ROWW].bitcast(dt.uint16))
            elif DUMP == "gbuf":
                glo, ghi, csl = chunks[0]
                q0 = int(offq[groups[glo][0]])
                gb = gp.tile([128, 2, csl, ROWW], dt.bfloat16, tag="gb")
                for r in range(2):
                    tab_view = table[0:T_HI, 0:ROWW] if r == 0 \
                        else table[T_LO:N, 0:ROWW]
                    idxr = idxA if r == 0 else idxB
                    nidx = csl * 128
                    nc.gpsimd.dma_gather(
                        out_ap=gb[:, r, :, :], in_ap=tab_view,
                        idxs_ap=idxr[:, q0 * 8:(q0 + csl) * 8],
                        num_idxs=nidx, num_idxs_reg=nidx,
                        elem_size=ROWW, elem_step=TABW, single_packet=False,
                        queue_num=next_queue())
                nc.sync.dma_start(
                    out=out_d[:],
                    in_=gb[:].rearrange("p r q w -> p (r q w)").bitcast(dt.uint16))
            if DUMP in ("table", "gbuf"):
                LN_eff = 0
            elif DUMP:
                LN_eff = 1
            else:
                LN_eff = LN
            dbg = None
            if DUMP in ("alpha", "ex", "agg"):
                _w = chunks[0][2] * (H if DUMP == "agg" else 1)
                dbg = res.tile([128, 2 * _w], dt.float32, name="dbg")

            # ---------------- layers ----------------
            for li in range(1, LN_eff + 1):
                own_prev = own_tabs[(li + 1) % 2]
                own_new = own_tabs[li % 2]
                last = (li == L_FULL)
                for (glo, ghi, csl) in chunks:
                    b0 = groups[glo][0]
                    q0 = int(offq[b0])
                    gb = gp.tile([128, 2, csl, ROWW], dt.bfloat16, tag="gb")
                    for r in range(2):
                        tab_view = table[0:T_HI, 0:ROWW] if r == 0 \
                            else table[T_LO:N, 0:ROWW]
                        idxr = idxA if r == 0 else idxB
                        nidx = csl * 128
                        nc.gpsimd.dma_gather(
                            out_ap=gb[:, r, :, :],
                            in_ap=tab_view,
                            idxs_ap=idxr[:, q0 * 8:(q0 + csl) * 8],
                            num_idxs=nidx, num_idxs_reg=nidx,
                            elem_size=ROWW, elem_step=TABW,
                            single_packet=False, queue_num=next_queue())
                    for gi in range(glo, ghi):
                        bg, G, D = groups[gi]
                        qa = int(offq[bg]) - q0
                        GD = G * D
                        S2 = 2 * GD
                        # views
                        hp_g = gb[:, :, qa:qa + GD, 0:H].rearrange(
                            "p r (g d) f -> p r g d f", g=G)
                        gf32 = gb[:].bitcast(dt.float32)
                        aL_g = gf32[:, :, qa:qa + GD, H // 2]       # [p,2,GD]
                        ownf = own_prev[:].bitcast(dt.float32)
                        aR_o = ownf[:, bg:bg + G, H // 2 + 1]       # [p,G]
                        hp_o = own_prev[:, bg:bg + G, 0:H]          # [p,G,32]

                        prod = wp.tile([128, S2, H], dt.bfloat16, tag="prod")
                        prodv = prod[:].rearrange("p (r q) f -> p r q f", r=2)
                        for r in range(2):
                            nc.vector.tensor_tensor(
                                out=prodv[:, r].rearrange(
                                    "p (g d) f -> p g d f", g=G),
                                in0=gb[:, r, qa:qa + GD, 0:H].rearrange(
                                    "p (g d) f -> p g d f", g=G),
                                in1=hp_o.unsqueeze(2)
                                .broadcast_to([128, G, D, H]),
                                op=mybir.AluOpType.mult)
                        logit = sp.tile([128, S2], dt.float32, tag="logit")
                        nc.vector.tensor_reduce(
                            out=logit[:], in_=prod[:],
                            axis=mybir.AxisListType.X, op=mybir.AluOpType.add,
                            negate=True)
                        sig = sp.tile([128, S2], dt.float32, tag="sig")
                        nc.scalar.activation(
                            out=sig[:], in_=logit[:],
                            func=mybir.ActivationFunctionType.Exp)
                        nc.vector.tensor_scalar(
                            out=sig[:], in0=sig[:], scalar1=1.0, scalar2=None,
                            op0=mybir.AluOpType.add)
                        nc.vector.reciprocal(out=sig[:], in_=sig[:])
                        alpha = sp.tile([128, S2], dt.float32, tag="alpha")
                        nc.vector.tensor_tensor(
                            out=alpha[:].rearrange("p (r g d) -> p r g d",
                                                   r=2, g=G),
                            in0=aL_g.rearrange("p r (g d) -> p r g d", g=G),
                            in1=aR_o.unsqueeze(1).unsqueeze(3).broadcast_to(
                                [128, 2, G, D]),
                            op=mybir.AluOpType.add)
                        nc.vector.tensor_tensor(out=alpha[:], in0=alpha[:],
                                                in1=sig[:],
                                                op=mybir.AluOpType.mult)
                        asc = sp.tile([128, S2], dt.float32, tag="asc")
                        nc.vector.tensor_scalar(
                            out=asc[:], in0=alpha[:], scalar1=NEG, scalar2=None,
                            op0=mybir.AluOpType.mult)
                        nc.vector.tensor_tensor(
                            out=alpha[:], in0=alpha[:], in1=asc[:],
                            op=mybir.AluOpType.max)
                        mk = maskr[:].rearrange("p (r q) -> p r q", r=2)[
                            :, :, qa + q0:qa + q0 + GD]
                        nc.vector.tensor_tensor(
                            out=alpha[:].rearrange("p (r q) -> p r q", r=2),
                            in0=alpha[:].rearrange("p (r q) -> p r q", r=2),
                            in1=mk, op=mybir.AluOpType.add)
                        if dbg is not None and li == 1 and glo == 0 and DUMP == "alpha":
                            nc.vector.tensor_copy(
                                out=dbg[:].rearrange("p (r q) -> p r q", r=2)[
                                    :, :, qa:qa + GD],
                                in_=alpha[:].rearrange("p (r q) -> p r q", r=2))
                        am2 = sp.tile([128, 2 * G], dt.float32, tag="am2")
                        nc.vector.tensor_reduce(
                            out=am2[:],
                            in_=alpha[:].rearrange("p (rg d) -> p rg d", d=D),
                            axis=mybir.AxisListType.X, op=mybir.AluOpType.max)
                        nam = sp.tile([128, G], dt.float32, tag="nam")
                        nc.vector.tensor_reduce(
                            out=nam[:],
                            in_=am2[:].rearrange("p (r g) -> p g r", r=2),
                            axis=mybir.AxisListType.X, op=mybir.AluOpType.max,
                            negate=True)
                        nc.vector.tensor_tensor(
                            out=alpha[:].rearrange("p (r g d) -> p r g d",
                                                   r=2, g=G),
                            in0=alpha[:].rearrange("p (r g d) -> p r g d",
                                                   r=2, g=G),
                            in1=nam[:].unsqueeze(1).unsqueeze(3).broadcast_to(
                                [128, 2, G, D]),
                            op=mybir.AluOpType.add)
                        ex = sp.tile([128, S2], dt.bfloat16, tag="ex")
                        nc.scalar.activation(
                            out=ex[:], in_=alpha[:],
                            func=mybir.ActivationFunctionType.Exp)
                        if dbg is not None and li == 1 and glo == 0 and DUMP == "ex":
                            nc.vector.tensor_copy(
                                out=dbg[:].rearrange("p (r q) -> p r q", r=2)[
                                    :, :, qa:qa + GD],
                                in_=ex[:].rearrange("p (r q) -> p r q", r=2))
                        den2 = sp.tile([128, 2 * G], dt.float32, tag="den2")
                        nc.vector.tensor_reduce(
                            out=den2[:],
                            in_=ex[:].rearrange("p (rg d) -> p rg d", d=D),
                            axis=mybir.AxisListType.X, op=mybir.AluOpType.add)
                        rden = sp.tile([128, G], dt.float32, tag="rden")
                        den1 = sp.tile([128, G], dt.float32, tag="den1")
                        nc.vector.tensor_reduce(
                            out=den1[:],
                            in_=den2[:].rearrange("p (r g) -> p g r", r=2),
                            axis=mybir.AxisListType.X, op=mybir.AluOpType.add)
                        nc.vector.reciprocal(out=rden[:], in_=den1[:])
                        wv = wp.tile([128, S2, H], dt.bfloat16, tag="wv")
                        nc.vector.tensor_tensor(
                            out=wv[:].rearrange("p (r q) f -> p r q f", r=2),
                            in0=gb[:, :, qa:qa + GD, 0:H],
                            in1=ex[:].rearrange("p (r q) -> p r q", r=2)
                            .unsqueeze(3).broadcast_to([128, 2, GD, H]),
                            op=mybir.AluOpType.mult)
                        agg2 = tp.tile([128, 2, G, H], dt.float32, tag="agg2")
                        nc.vector.tensor_reduce(
                            out=agg2[:].rearrange("p r g f -> p (r g) f"),
                            in_=wv[:].rearrange(
                                "p (r g d) f -> p (r g) f d", r=2, g=G),
                            axis=mybir.AxisListType.X,
                            op=mybir.AluOpType.add)
                        agg = tp.tile([128, G, H], dt.float32, tag="agg")
                        nc.vector.tensor_tensor(
                            out=agg[:], in0=agg2[:, 0], in1=agg2[:, 1],
                            op=mybir.AluOpType.add)
                        nc.vector.tensor_tensor(
                            out=agg[:], in0=agg[:],
                            in1=rden[:].unsqueeze(2).broadcast_to([128, G, H]),
                            op=mybir.AluOpType.mult)
                        if dbg is not None and li == 1 and glo == 0 and DUMP == "agg":
                            nc.vector.tensor_copy(
                                out=dbg[:, qa * H:(qa + G) * H],
                                in_=agg[:].rearrange("p g h -> p (g h)"))
                        nc.vector.tensor_tensor(
                            out=agg[:], in0=agg[:],
                            in1=brep[:].rearrange("p (l h) -> p l h",
                                                  l=L_FULL)[:, li - 1]
                            .unsqueeze(1).broadcast_to([128, G, H]),
                            op=mybir.AluOpType.add)
                        hnext = tp.tile([128, G, H], dt.float32, tag="hnext")
                        nc.vector.tensor_scalar(
                            out=hnext[:], in0=agg[:], scalar1=0.0, scalar2=None,
                            op0=mybir.AluOpType.max)
                        # tails per block
                        wslice = waug[:].rearrange(
                            "h (l w) -> h l w", l=L_FULL)[:, li - 1, :]
                        for gg in range(G):
                            b = bg + gg
                            hT_ps = pt.tile([H, 128], dt.float32,
                                            space="PSUM", tag="hT")
                            nc.tensor.transpose(out=hT_ps[:],
                                                in_=hnext[:, gg, :],
                                                identity=ident[:])
                            hT = sp.tile([H, 128], dt.float32, tag="hTs")
                            nc.scalar.copy(out=hT[:], in_=hT_ps[:])
                            mm = pm.tile([128, ROWW], dt.float32,
                                         space="PSUM", tag="mm")
                            if last:
                                nc.tensor.matmul(out=mm[:, 0:D_OUT],
                                                 lhsT=hT[:],
                                                 rhs=wslice[:, 0:D_OUT],
                                                 start=True, stop=True)
                                nc.vector.tensor_tensor(
                                    out=outstage[:, b, :],
                                    in0=mm[:, 0:D_OUT], in1=b16r[:],
                                    op=mybir.AluOpType.add)
                            else:
                                nc.tensor.matmul(out=mm[:], lhsT=hT[:],
                                                 rhs=wslice[:],
                                                 start=True, stop=True)
                                pack_row(mm[:], own_new, b)
                if dbg is not None and li == 1:
                    nc.sync.dma_start(out=out_d[:], in_=dbg[:])
                    break
                if last:
                    nc.sync.dma_start(
                        out=out_d[:].rearrange("(b p) w -> p b w", p=128),
                        in_=outstage[:])
                elif li == LN:
                    nc.sync.dma_start(
                        out=out_d[:].rearrange("(b p) w -> p b w", p=128),
                        in_=own_new[:].bitcast(dt.uint16))
                else:
                    table = exchange(own_new, li)

    nc.compile()
    return nc


# ----------------------------------------------------------------------------
# entry point
# ----------------------------------------------------------------------------

_CACHE = {}
LAST_RESULT = None


def kernel(x, edge_index, W0, b0, Ws, att_l, att_r, bs, W16, b16):
    global LAST_RESULT
    x = np.asarray(x, dtype=np.float32)
    edge_index = np.asarray(edge_index)
    pre = _preprocess(edge_index)
    wts = _prep_weights(np.asarray(W0, np.float32), np.asarray(b0, np.float32),
                        np.asarray(Ws, np.float32),
                        np.asarray(att_l, np.float32),
                        np.asarray(att_r, np.float32),
                        np.asarray(bs, np.float32),
                        np.asarray(W16, np.float32),
                        np.asarray(b16, np.float32))
    key = pre["key"]
    if key not in _CACHE:
        _CACHE[key] = _build_program(pre["sched"])
    nc = _CACHE[key]

    inv_perm = pre["inv_perm"]
    in_maps = []
    for c in range(NC):
        pids = np.arange(c * NPC, (c + 1) * NPC)
        orig = inv_perm[pids]
        xT = np.zeros((D_IN, NPAD), np.float32)
        xT[:, 0:NPC] = x[orig].T
        in_maps.append(dict(
            xT=xT, idxA=pre["idxA"][c], idxB=pre["idxB"][c],
            mask=pre["mask"][c].reshape(128, -1),
            wfold=wts["wfold"], bfold=wts["bfold"], waug=wts["waug"],
            brep=wts["brep"], b16rep=wts["b16rep"]))

    res = bass_utils.run_bass_kernel_spmd(
        nc, in_maps, core_ids=list(range(NC)),
        tmpdir=os.environ.get("SGAT_TMPDIR") or None)
    LAST_RESULT = res

    if L_DEBUG >= L_FULL:
        out = np.zeros((N, D_OUT), np.float32)
        for c in range(NC):
            pids = np.arange(c * NPC, (c + 1) * NPC)
            out[inv_perm[pids]] = res.results[c]["out"][0:NPC]
        return out
    else:
        # debug: return raw table_{L+1} rows per permuted id
        out = np.zeros((N, ROWW), np.uint16)
        for c in range(NC):
            pids = np.arange(c * NPC, (c + 1) * NPC)
            out[inv_perm[pids]] = res.results[c]["out"][0:NPC]
        return out

